# revision 61
# baseline (speedup 1.0000x reference)
"""MoE transformer block on 8 Trainium2 cores.

Layer: x = x + attn(ln1(x)); x = x + moe(ln2(x)).
Shapes: B=4, T=1024, C=768, H=12 heads, E=8 experts, top-2, cap=1280, F=3072.

Distribution:
  Launch A (attention, bf16): core i -> batch i//2, heads 6*(i%2) .. +6.
    LN1 (affine folded into the QKV weights) is applied host-side and x-hat
    arrives pre-transposed [C, T]. The head loop is software-pipelined depth
    2 (AV of head h trails scores of head h+2) so the softmax-exp stream on
    the Activation engine — the middle-phase critical path — never starves.
    Each core emits a partial (6-head) projection, transposed [C, T] bf16;
    host sums the two half-head partials per batch and adds the residual.
  Host: ln2 + gating + exact top-2 capacity routing (numpy, matches the jax
    reference in ordering; near-tie tokens get exact fp32 logits), builds
    per-expert gather indices.
  Launch B (experts, fp8): core e -> expert e, slots packed to min(observed
    max load rounded to 64, 1024). Both matmuls run fp8(e4m3) DoubleRow
    (K=256/instr, 0.5 cyc/row); weights are quantized per 4-mf group
    host-side, activations are cast directly (|x|<5 fits e4m3), dequant
    rides the PSUM-drain ops. Token dim is chunked 4x256 with mm2 pieces
    woven into the mm1/gelu stream. outT [C, cap_k] bf16; host scatter-adds
    w * out into y and computes overflow slots (beyond cap_k) in fp32.
"""

import math

import numpy as np
import ml_dtypes

import concourse.bacc as bacc
import concourse.bass as bass
import concourse.mybir as mybir
import concourse.tile as tile
from concourse import bass_utils
from concourse.masks import make_identity

F32 = mybir.dt.float32
BF16 = mybir.dt.bfloat16
FP8 = mybir.dt.float8e4
E4 = ml_dtypes.float8_e4m3  # matches TRN float8e4 (max ±240)
AF = mybir.ActivationFunctionType
ALU = mybir.AluOpType
AX = mybir.AxisListType

B, T, C = 4, 1024, 768
NHEAD = 12
HD = C // NHEAD  # 64
E = 8
TOPK = 2
CAP = 1280
F = 4 * C  # 3072
LN_EPS = 1e-5
NEG_INF = -1e30
P = 128

N_CORES = 8
H6 = NHEAD // 2          # heads per core
D6 = H6 * HD             # 384
CSUB = C // P            # 6
KSUB_F = F // P          # 24
NT = T // P              # 8
QKV9 = 3 * D6 // P       # 9

_CACHE = {}


def _chunks(n, step=512):
    out = []
    s = 0
    while s < n:
        out.append((s, min(step, n - s)))
        s += step
    return out


def _run_spmd(nc, in_maps):
    """run_bass_kernel_spmd with one retry (transient NRT/axon failures)."""
    try:
        return bass_utils.run_bass_kernel_spmd(
            nc, in_maps, core_ids=list(range(N_CORES)))
    except Exception:
        import time as _time
        _time.sleep(2.0)
        return bass_utils.run_bass_kernel_spmd(
            nc, in_maps, core_ids=list(range(N_CORES)))


# --------------------------------------------------------------------------
# Launch A: attention
# --------------------------------------------------------------------------

def build_attn():
    nc = bacc.Bacc("TRN2", target_bir_lowering=False, debug=False)

    # LN1 is applied host-side (host already computes the stats); the kernel
    # receives x-hat transposed [C, T] so the qkv matmuls start immediately.
    xlnTd = nc.dram_tensor("xlnT", [P, CSUB, T], BF16, kind="ExternalInput")
    # qkv weight slice for this core's 6 heads, ln1-folded, q pre-scaled by
    # 1/sqrt(HD), grouped (0,3,6),(1,4,7),(2,5,8) — the order the head loop
    # consumes the 128-col groups. column order within n:
    # q h0..h5 | k h0..h5 | v h0..h5 (64 cols each head)
    wqkv = nc.dram_tensor("wqkv", [P, QKV9 * CSUB * P], BF16,
                          kind="ExternalInput")
    wpj = nc.dram_tensor("wpj", [P, D6 // P, C], BF16, kind="ExternalInput")
    # transposed causal 0/1 mask (bf16): cmaskT[k, q] = 1 if k <= q else 0
    cmaskT = nc.dram_tensor("cmaskT", [P, P], BF16, kind="ExternalInput")
    # packed [bqkv(9) | bpj(6)] — one DMA; HWDGE setup is ~0.6us per copy,
    # so copy count gates the lead-in
    meta = nc.dram_tensor("meta", [P, QKV9 + CSUB], F32, kind="ExternalInput")
    out = nc.dram_tensor("attn_pT", [C, T], BF16, kind="ExternalOutput")

    with tile.TileContext(nc) as tc:
        with (
            tc.tile_pool(name="const", bufs=1) as const,
            tc.tile_pool(name="big", bufs=1) as big,
            tc.tile_pool(name="pTp", bufs=5) as pTp,
            tc.tile_pool(name="work", bufs=4) as work,
            tc.tile_pool(name="ps_sc", bufs=2, space="PSUM") as ps_sc,
            tc.tile_pool(name="ps_mm", bufs=1, space="PSUM") as ps_mm,
            tc.tile_pool(name="ps_t", bufs=1, space="PSUM") as ps_t,
            tc.tile_pool(name="ps_y", bufs=2, space="PSUM") as ps_y,
        ):
            # PE warmup during the DMA lead-in (p-state ramp)
            wz = const.tile([P, 512], BF16, name="wz")
            nc.vector.memset(wz[:], 0.0)
            for wi in range(12):
                pw = ps_mm.tile([P, 512], F32, tag="mm", name=f"warm{wi}")
                nc.tensor.matmul(pw[:], lhsT=wz[:, :P], rhs=wz[:],
                                 start=True, stop=True)

            meta_sb = const.tile([P, QKV9 + CSUB], F32)
            nc.sync.dma_start(meta_sb[:], meta[:])
            bqkv_sb = meta_sb[:, 0:QKV9]
            bpj_sb = meta_sb[:, QKV9:]
            cm = const.tile([P, P], BF16)
            nc.sync.dma_start(cm[:], cmaskT[:])
            xlnT = [big.tile([P, CSUB, T // 2], BF16, name=f"xlnT{i}")
                    for i in range(2)]
            wq_g = [const.tile([P, 3, CSUB, P], BF16, name=f"wqg{gi}")
                    for gi in range(3)]

            def dma_wq(gi):
                blk = 3 * CSUB * P
                nc.sync.dma_start(
                    wq_g[gi][:].rearrange("p a b c -> p (a b c)"),
                    wqkv[:, gi * blk:(gi + 1) * blk])

            nc.sync.dma_start(xlnT[0][:], xlnTd[:, :, 0:T // 2])
            dma_wq(0)
            nc.sync.dma_start(xlnT[1][:], xlnTd[:, :, T // 2:T])
            dma_wq(1)
            dma_wq(2)
            ident = const.tile([P, P], BF16)
            make_identity(nc, ident[:])
            wpj_sb = const.tile([P, D6 // P, C], BF16)
            nc.sync.dma_start(wpj_sb[:], wpj[:])

            # ---- qkvT [3*D6, T] = wqkv.T @ xln.T, + bias
            # one SBUF tile per 128-row group so consumers wait only on the
            # rows they read
            qkvT = [big.tile([P, T], BF16, tag=f"qkvT{mc}", name=f"qkvT{mc}")
                    for mc in range(QKV9)]
            v_ones = big.tile([P, NT, H6, 1 + 64], BF16)
            nc.vector.memset(v_ones[:, :, :, 0:1], 1.0)
            y_big = big.tile([P, NT, D6], BF16)

            def emit_qkv(mc, use_sc=False):
                for th in range(T // 512):
                    # before any scores exist the sc ring is idle — borrow
                    # it for every other early chunk so the mm ring's
                    # drain-read latency never bubbles the PE
                    if use_sc and th % 2 == 1:
                        pacc = ps_sc.tile([P, 512], F32, tag="sc",
                                          name=f"qk{mc}{th}")
                    else:
                        pacc = ps_mm.tile([P, 512], F32, tag="mm",
                                          name=f"qk{mc}{th}")
                    for ks in range(CSUB):
                        nc.tensor.matmul(
                            pacc[:, :512],
                            lhsT=wq_g[mc % 3][:, mc // 3, ks, :],
                            rhs=xlnT[th][:, ks, :],
                            start=(ks == 0), stop=(ks == CSUB - 1))
                    # bias+copy PSUM->SBUF (PSUM is DVE/ACT-only)
                    nc.vector.tensor_scalar_add(
                        qkvT[mc][:, th * 512:(th + 1) * 512], pacc[:, :512],
                        bqkv_sb[:, mc:mc + 1])

            def emit_vones(j):
                # vT row j -> v for heads 2j, 2j+1 (col 0 stays all-ones).
                # All 8 transposes land in one PSUM tile, drained by a single
                # Pool copy, so the PE rips through without ring round-trips.
                pt8 = ps_t.tile([P, NT, P], BF16, tag="pt8", name=f"vt{j}")
                for ti in range(NT):
                    nc.tensor.transpose(
                        pt8[:, ti, :],
                        qkvT[2 * (D6 // P) + j][:, ti * P:(ti + 1) * P],
                        ident[:])
                nc.vector.tensor_copy(
                    v_ones[:, :, 2 * j:2 * j + 2, 1:],
                    pt8[:].rearrange("p t (a b) -> p t a b", a=2))

            def emit_scores(h, pT):
                # scores transposed sT[k, q] so Exp lands pT in SBUF directly;
                # the causal mask of the diagonal block is added by the PE.
                qp0 = 64 * (h % 2)
                qrow = h // 2
                kp0 = (D6 + 64 * h) % P
                krow = (D6 + 64 * h) // P
                for kb in range(NT):
                    q0 = kb * P
                    pscore = ps_sc.tile([P, T], F32, tag="sc",
                                        name=f"sc{h}{kb}")
                    # chunk on absolute 512 boundaries (PSUM bank alignment
                    # for the matmuls); Exp drains the whole row in one op
                    bounds = [q0] + [b for b in (512, T) if b > q0]
                    for (s0, e0) in zip(bounds[:-1], bounds[1:]):
                        w = e0 - s0
                        nc.tensor.matmul(
                            pscore[:, s0:s0 + w],
                            lhsT=qkvT[krow][kp0:kp0 + 64, kb * P:(kb + 1) * P],
                            rhs=qkvT[qrow][qp0:qp0 + 64, s0:s0 + w],
                            start=True, stop=True)
                    nc.scalar.activation(
                        pT[:, kb, q0:], pscore[:, q0:], AF.Exp)
                    # causal mask of the diagonal block: 0/1 multiply on the
                    # near-idle Pool engine (SBUF-only op) instead of a PE
                    # matmul add — exp of the unmasked scores stays finite
                    # in bf16 (|s| < ~20)
                    nc.gpsimd.tensor_tensor(
                        pT[:, kb, q0:q0 + P], pT[:, kb, q0:q0 + P],
                        cm[:], op=ALU.mult)

            yT = [big.tile([P, D6 // P, T // 2], BF16, tag=f"yT{i}",
                           name=f"yT{i}") for i in range(2)]

            def emit_av(h, pT):
                # AV for head h, softmax denominator fused via v_ones col 0;
                # normalization batched per qi-pair on DVE
                for q2 in range(NT // 2):
                    py = ps_y.tile([P, 2, 65], F32, tag="py",
                                   name=f"py{h}{q2}")
                    for qq in range(2):
                        qi = 2 * q2 + qq
                        for kb in range(qi + 1):
                            nc.tensor.matmul(
                                py[:, qq, :],
                                lhsT=pT[:, kb, qi * P:(qi + 1) * P],
                                rhs=v_ones[:, kb, h, :],
                                start=(kb == 0), stop=(kb == qi))
                    rec = work.tile([P, 2], F32, tag="rec")
                    nc.vector.reciprocal(rec[:], py[:, :, 0:1])
                    nc.vector.tensor_tensor(
                        y_big[:, 2 * q2:2 * q2 + 2, h * 64:(h + 1) * 64],
                        py[:, :, 1:],
                        rec[:].to_broadcast([P, 2, 64]), op=ALU.mult)

            def emit_yt(g):
                # batched yT transpose for the 128-row block of heads 2g/2g+1
                pt8 = ps_t.tile([P, NT, P], BF16, tag="pt8", name=f"yt{g}")
                for qi in range(NT):
                    nc.tensor.transpose(
                        pt8[:, qi, :], y_big[:, qi, g * P:(g + 1) * P],
                        ident[:])
                for i in range(2):
                    nc.vector.tensor_copy(
                        yT[i][:, g, :], pt8[:, 4 * i:4 * i + 4, :]
                        .rearrange("p a b -> p (a b)"))

            # head-level software pipelining, depth 2: AV(h) trails
            # scores(h+2), so the exp stream on ACT (the middle-phase
            # critical path) never starves while the in-order PE waits on
            # exp-gated AV work
            pTs = []

            def emit_block(h):
                g = h // 2
                if h % 2 == 0:
                    emit_qkv(g, use_sc=(h == 0))
                    emit_qkv(3 + g, use_sc=(h == 0))
                else:
                    emit_qkv(6 + g)
                    emit_vones(g)
                pT = pTp.tile([P, NT, T], BF16, tag="pT", name=f"pT{h}")
                emit_scores(h, pT)
                pTs.append(pT)

            emit_block(0)
            emit_block(1)
            emit_block(2)
            for h in range(H6):
                if h + 3 < H6:
                    emit_block(h + 3)
                emit_av(h, pTs[h])
                if h % 2 == 1:
                    emit_yt(h // 2)

            # ---- partial projection: outT [C, T] = wpj.T @ y.T + bpj
            # pacc alternates between the two PSUM rings so chunks pipeline
            for cc in range(CSUB):
                o_sb = work.tile([P, T], BF16, tag="osb")
                for th in range(T // 512):
                    pool = ps_mm if (2 * cc + th) % 2 == 0 else ps_sc
                    tag = "mm" if pool is ps_mm else "sc"
                    pacc = pool.tile([P, 512], F32, tag=tag,
                                     name=f"pj{cc}{th}")
                    for j in range(D6 // P):
                        nc.tensor.matmul(
                            pacc[:, :512],
                            lhsT=wpj_sb[:, j, cc * P:(cc + 1) * P],
                            rhs=yT[th][:, j, :],
                            start=(j == 0), stop=(j == D6 // P - 1))
                    nc.vector.tensor_scalar_add(
                        o_sb[:, th * 512:(th + 1) * 512], pacc[:, :512],
                        bpj_sb[:, cc:cc + 1])
                nc.sync.dma_start(out[cc * P:(cc + 1) * P, :], o_sb[:])

    nc.compile()
    return nc


# --------------------------------------------------------------------------
# Launch B: experts
# --------------------------------------------------------------------------

def build_expert(cap_k, grouped=True):
    """fp8(e4m3) expert MLP. Both matmuls run in DoubleRow perf mode (K=256
    per instruction, 0.5 cyc/row). Weights are quantized host-side with
    scales shared per 4-mf group; dequant rides the PSUM-drain op. hT stays
    fp8 in SBUF (mm2's rhs must be fp8). Token dim is processed in 256-col
    chunks, interleaving mm1/mm2 chunks on the (in-order) PE so mm2 work
    hides behind the gelu stream on ACT — which is the pacing engine.
    `grouped` (one ACT op per 4-mf [P,4,256] PSUM region) requires
    group-equal biases; fallback is one gelu per mf."""
    nc = bacc.Bacc("TRN2", target_bir_lowering=False, debug=False)

    G = 4                    # mf group per gelu op
    NQ = KSUB_F // G         # 6 groups
    xbT = nc.dram_tensor("xbT", [P, CSUB, cap_k], FP8, kind="ExternalInput")
    fcw = nc.dram_tensor("fcw", [P, KSUB_F * CSUB * P], FP8,
                         kind="ExternalInput")
    nsc = NQ if grouped else KSUB_F
    # packed [fcs | fcb | pjs | pjb] — one DMA
    scb = nc.dram_tensor("scb", [P, 2 * nsc + 2 * CSUB], F32,
                         kind="ExternalInput")
    pjw = nc.dram_tensor("pjw", [P, CSUB * KSUB_F * P], FP8,
                         kind="ExternalInput")
    out = nc.dram_tensor("outT", [C, cap_k], BF16, kind="ExternalOutput")

    SC = _chunks(cap_k, 256)          # compute chunks
    SD = _chunks(cap_k, 512)          # xbT DMA pieces (512B runs)
    DR = mybir.MatmulPerfMode.DoubleRow
    MFBLK = CSUB * P                  # 768 fp8 bytes per mf per partition

    NCH = len(SC)
    with tile.TileContext(nc) as tc:
        with (
            tc.tile_pool(name="const", bufs=1) as const,
            tc.tile_pool(name="osb", bufs=4) as osbp,
            tc.tile_pool(name="ps1", bufs=2, space="PSUM") as ps1,
            tc.tile_pool(name="ps2", bufs=3, space="PSUM") as ps2,
            tc.tile_pool(name="psw", bufs=1, space="PSUM") as psw,
        ):
            # PE warmup during the xbT/weight DMA lead-in
            wz = const.tile([P, 512], BF16, name="wz")
            nc.vector.memset(wz[:], 0.0)
            for wi in range(6):
                pw = psw.tile([P, 512], F32, tag="warm", name=f"warm{wi}")
                nc.tensor.matmul(pw[:], lhsT=wz[:, :P], rhs=wz[:],
                                 start=True, stop=True)

            # NOTE: tile-granular dependency tracking — every dma/compute
            # producer gets its own tile so consumers wait only on what they
            # actually read. One packed tile for the four tiny scale/bias
            # vectors (single DMA; HWDGE setup is ~0.6us per copy).
            sc_sb = const.tile([P, 2 * nsc + 2 * CSUB], F32)
            fcs_sb = sc_sb[:, :nsc]
            fcb_sb = sc_sb[:, nsc:2 * nsc]
            pjs_sb = sc_sb[:, 2 * nsc:2 * nsc + CSUB]
            pjb_sb = sc_sb[:, 2 * nsc + CSUB:]

            xb_t = [const.tile([P, CSUB, sw], FP8, name=f"xb{i}")
                    for i, (s0, sw) in enumerate(SD)]
            # quad 0 split in two 2-mf tiles: its DMA gates the very first
            # matmul, so halving the first transfer shaves the lead-in
            w1_q0 = [const.tile([P, 2, CSUB, P], FP8, name=f"w1q0{h}")
                     for h in range(2)]
            w1_t = [const.tile([P, G, CSUB, P], FP8, name=f"w1q{q}")
                    for q in range(1, NQ)]

            def w1_lhsT(q, g):
                if q == 0:
                    return w1_q0[g // 2][:, g % 2]
                return w1_t[q - 1][:, g]
            w2_t = [const.tile([P, KSUB_F, P], FP8, name=f"w2c{cc}")
                    for cc in range(CSUB)]
            hT_t = [const.tile([P, KSUB_F, sw], FP8, name=f"hT{i}")
                    for i, (s0, sw) in enumerate(SC)]

            def dma_xbT(i):
                s0, sw = SD[i]
                nc.sync.dma_start(xb_t[i][:], xbT[:, :, s0:s0 + sw])

            def dma_w1(q):
                if q == 0:
                    for h in range(2):
                        nc.sync.dma_start(
                            w1_q0[h][:].rearrange("p a b c -> p (a b c)"),
                            fcw[:, 2 * h * MFBLK:2 * (h + 1) * MFBLK])
                    return
                nc.sync.dma_start(
                    w1_t[q - 1][:].rearrange("p a b c -> p (a b c)"),
                    fcw[:, q * G * MFBLK:(q + 1) * G * MFBLK])

            def dma_w2(cc):
                blk = KSUB_F * P
                nc.sync.dma_start(
                    w2_t[cc][:].rearrange("p a b -> p (a b)"),
                    pjw[:, cc * blk:(cc + 1) * blk])

            # ordered by first consumer: xbT piece 0 and all w1 first (they
            # pace mm1(c0) and with it the whole gelu stream), then w2
            # staged around the late-needed xbT tail
            dma_xbT(0)
            dma_w1(0)
            nc.sync.dma_start(sc_sb[:], scb[:])
            for q in range(1, NQ):
                dma_w1(q)
            dma_w2(0)
            dma_w2(1)
            dma_w2(2)
            for i in range(1, len(SD)):
                dma_xbT(i)
            dma_w2(3)
            dma_w2(4)
            dma_w2(5)

            def piece_of(s0):
                for i, (p0, pw) in enumerate(SD):
                    if p0 <= s0 < p0 + pw:
                        return i, s0 - p0
                raise AssertionError(s0)

            def mm1(ci):
                s0, sw = SC[ci]
                di, d0 = piece_of(s0)
                for q in range(NQ):
                    if ci == 0:
                        # filler: the first chunk is paced by the w1 quad
                        # DMAs; keep the PE p-state warm across the ~0.4us
                        # per-quad wait
                        pw = psw.tile([P, 512], F32, tag="warm",
                                      name=f"fill{q}")
                        nc.tensor.matmul(pw[:], lhsT=wz[:, :P], rhs=wz[:],
                                         start=True, stop=True)
                    pacc = ps1.tile([P, G, 256], F32, tag="mm1")
                    for g in range(G):
                        for j in range(CSUB // 2):
                            nc.tensor.matmul(
                                pacc[:, g, :sw],
                                lhsT=w1_lhsT(q, g)[:, 2 * j:2 * j + 2, :],
                                rhs=xb_t[di][:, 2 * j:2 * j + 2, d0:d0 + sw],
                                start=(j == 0), stop=(j == CSUB // 2 - 1),
                                perf_mode=DR)
                    if grouped:
                        nc.scalar.activation(
                            hT_t[ci][:, G * q:G * q + G, :sw],
                            pacc[:, :, :sw], AF.Gelu,
                            bias=fcb_sb[:, q:q + 1],
                            scale=fcs_sb[:, q:q + 1])
                    else:
                        for g in range(G):
                            mf = G * q + g
                            nc.scalar.activation(
                                hT_t[ci][:, mf, :sw],
                                pacc[:, g, :sw], AF.Gelu,
                                bias=fcb_sb[:, mf:mf + 1],
                                scale=fcs_sb[:, mf:mf + 1])

            def mm2(ci, cc):
                # one (chunk, output-c-tile) piece: parks the in-order PE
                # only on its own w2 tile / hT chunk
                s0, sw = SC[ci]
                pacc = ps2.tile([P, 256], F32, tag="mm2")
                for j in range(KSUB_F // 2):
                    nc.tensor.matmul(
                        pacc[:, :sw],
                        lhsT=w2_t[cc][:, 2 * j:2 * j + 2, :],
                        rhs=hT_t[ci][:, 2 * j:2 * j + 2, :sw],
                        start=(j == 0), stop=(j == KSUB_F // 2 - 1),
                        perf_mode=DR)
                # dequant+bias on DVE; ACT is saturated by mm1's gelu
                o_sb = osbp.tile([P, 256], BF16, tag="osb")
                nc.vector.tensor_scalar(
                    o_sb[:, :sw], pacc[:, :sw],
                    pjs_sb[:, cc:cc + 1], pjb_sb[:, cc:cc + 1],
                    op0=ALU.mult, op1=ALU.add)
                nc.sync.dma_start(
                    out[cc * P:(cc + 1) * P, s0:s0 + sw], o_sb[:, :sw])

            # PE program order: the gelu stream on ACT (1.09us per quad-op)
            # paces the pipeline; mm1(c) quads are throttled to it by the
            # ps1 rotation, so weave the previous chunk's mm2 pieces into
            # the stream to keep the in-order PE from idling
            mm1(0)
            for i in range(1, NCH):
                mm1(i)
                for cc in range(CSUB):
                    mm2(i - 1, cc)
            for cc in range(CSUB):
                mm2(NCH - 1, cc)

    nc.compile()
    return nc


# --------------------------------------------------------------------------
# Host glue
# --------------------------------------------------------------------------

def _bf16(a):
    return np.asarray(a, np.float32).astype(ml_dtypes.bfloat16)


def _pcol(vec, nsub):
    """[nsub*P] -> [P, nsub] per-partition bias layout."""
    return np.ascontiguousarray(
        np.asarray(vec, np.float32).reshape(nsub, P).T)


def _kperm(w):
    """[K, N] -> [P, K//P, N] partition-major layout, contiguous."""
    k, n = w.shape
    return np.ascontiguousarray(w.reshape(k // P, P, n).transpose(1, 0, 2))


def _layer_norm(x, w, b):
    mu = x.mean(-1, keepdims=True)
    var = x.var(-1, keepdims=True)
    return (x - mu) / np.sqrt(var + LN_EPS) * w + b


def _exact_logits(need, x, ln1_w, ln1_b, ln2_w, ln2_b, qkv_w, qkv_b,
                  proj_w, proj_b, w_g):
    """fp32 gating logits for the given flat token indices (exact attention
    rows for just those tokens)."""
    out = np.empty((need.size, E), np.float32)
    bs, ps = need // T, need % T
    for b in np.unique(bs):
        m = bs == b
        pos = ps[m]                              # [M]
        xl = _layer_norm(x[b], ln1_w, ln1_b)     # [T, C]
        kv = xl @ qkv_w[:, C:] + qkv_b[C:]       # [T, 2C]
        k = kv[:, :C].reshape(T, NHEAD, HD)
        v = kv[:, C:].reshape(T, NHEAD, HD)
        q = (xl[pos] @ qkv_w[:, :C] + qkv_b[:C]).reshape(-1, NHEAD, HD)
        s = np.einsum("mhd,khd->mhk", q, k) / math.sqrt(HD)
        s = np.where(pos[:, None, None] >= np.arange(T)[None, None, :],
                     s, NEG_INF)
        s -= s.max(-1, keepdims=True)
        p = np.exp(s)
        p /= p.sum(-1, keepdims=True)
        y = np.einsum("mhk,khd->mhd", p, v).reshape(-1, C)
        att = y @ proj_w + proj_b
        x2 = x[b][pos] + att
        out[m] = _layer_norm(x2, ln2_w, ln2_b) @ w_g
    return out


def kernel(x, ln1_w, ln1_b, ln2_w, ln2_b, attn_qkv_w, attn_qkv_b,
           attn_proj_w, attn_proj_b, w_g, exp_fc_w, exp_fc_b,
           exp_proj_w, exp_proj_b):
    x = np.asarray(x, np.float32)
    ln1_w = np.asarray(ln1_w, np.float32)
    ln1_b = np.asarray(ln1_b, np.float32)
    attn_qkv_w = np.asarray(attn_qkv_w, np.float32)
    attn_qkv_b = np.asarray(attn_qkv_b, np.float32)
    attn_proj_w = np.asarray(attn_proj_w, np.float32)
    attn_proj_b = np.asarray(attn_proj_b, np.float32)

    if "attn" not in _CACHE:
        _CACHE["attn"] = build_attn()

    # ---------------- launch A ----------------
    # fold ln1 affine into qkv: qkv = xhat @ (diag(w1) W) + (b1 @ W + b)
    Wf = ln1_w[:, None] * attn_qkv_w          # [C, 3C]
    bf = ln1_b @ attn_qkv_w + attn_qkv_b      # [3C]
    Wq = Wf[:, :C] / math.sqrt(HD)
    bq = bf[:C] / math.sqrt(HD)
    Wk, bk = Wf[:, C:2 * C], bf[C:2 * C]
    Wv, bv = Wf[:, 2 * C:], bf[2 * C:]

    cmaskT_np = _bf16(np.triu(np.ones((P, P), np.float32)))

    in_maps_a = []
    for core in range(N_CORES):
        b = core // 2
        h0 = H6 * (core % 2)
        cols = slice(h0 * HD, (h0 + H6) * HD)
        wqkv_c = np.concatenate([Wq[:, cols], Wk[:, cols], Wv[:, cols]], 1)
        bqkv_c = np.concatenate([bq[cols], bk[cols], bv[cols]])
        bpj_c = attn_proj_b if core % 2 == 0 else np.zeros(C, np.float32)
        mu_b = x[b].mean(-1)
        rstd_b = 1.0 / np.sqrt(x[b].var(-1) + LN_EPS)
        xln_b = (x[b] - mu_b[:, None]) * rstd_b[:, None]    # [T, C]
        xlnT_h = _bf16(xln_b).T.reshape(CSUB, P, T).transpose(1, 0, 2)
        wqkv_m = _bf16(wqkv_c).reshape(CSUB, P, QKV9, P)
        wqkv_m = wqkv_m.transpose(1, 2, 0, 3)[:, [0, 3, 6, 1, 4, 7, 2, 5, 8]]
        meta = np.concatenate([
            _pcol(bqkv_c, QKV9), _pcol(bpj_c, CSUB)], axis=1)
        in_maps_a.append({
            "xlnT": np.ascontiguousarray(xlnT_h),
            "meta": np.ascontiguousarray(meta.astype(np.float32)),
            "wqkv": np.ascontiguousarray(wqkv_m.reshape(P, -1)),
            "wpj": _kperm(_bf16(attn_proj_w[h0 * HD:(h0 + H6) * HD, :])),
            "cmaskT": cmaskT_np,
        })

    res_a = _run_spmd(_CACHE["attn"], in_maps_a)

    attn = np.empty((B, T, C), np.float32)
    for b in range(B):
        attn[b] = (np.asarray(res_a.results[2 * b]["attn_pT"], np.float32)
                   + np.asarray(res_a.results[2 * b + 1]["attn_pT"],
                                np.float32)).T

    x2 = x + attn                       # [B, T, C]
    xf2 = x2.reshape(B * T, C)

    # ---------------- host routing (exact reference semantics) -------------
    N = B * T
    xln2 = _layer_norm(xf2, np.asarray(ln2_w, np.float32),
                       np.asarray(ln2_b, np.float32))
    logits = xln2 @ np.asarray(w_g, np.float32)        # [N, E]

    # The top-2 expert choice is discontinuous: tokens whose top2/top3 gating
    # logits are within the bf16 noise floor could route differently than the
    # fp32 reference would. Recompute those few tokens' logits exactly.
    srt = np.sort(logits, axis=1)
    need = np.nonzero(srt[:, -2] - srt[:, -3] < 0.02)[0]
    if need.size:
        logits[need] = _exact_logits(
            need, x, ln1_w, ln1_b, np.asarray(ln2_w, np.float32),
            np.asarray(ln2_b, np.float32), attn_qkv_w, attn_qkv_b,
            attn_proj_w, attn_proj_b, np.asarray(w_g, np.float32))

    order = np.argsort(-logits, axis=1, kind="stable")
    topk_idx = order[:, :TOPK]                          # [N, K]
    sel = np.zeros((N, E), bool)
    np.put_along_axis(sel, topk_idx, True, axis=1)
    masked = np.where(sel, logits, NEG_INF)
    m = masked.max(1, keepdims=True)
    ex = np.exp(masked - m)
    router_probs = ex / ex.sum(1, keepdims=True)        # [N, E]

    # capacity ranks in (k, n) order
    exp_mask = np.zeros((TOPK, N, E), np.int64)
    kk = np.arange(TOPK)[:, None]
    nn = np.arange(N)[None, :]
    exp_mask[kk, nn, topk_idx.T] = 1
    flat = exp_mask.reshape(TOPK * N, E)
    rank = np.cumsum(flat, axis=0) - 1                  # [K*N, E]
    keep = (flat == 1) & (rank < CAP)
    kpos, epos = np.nonzero(keep)
    token = kpos % N
    slot = rank[kpos, epos]
    wgt = router_probs[token, epos]

    # pack the expert batches to the observed max load; if only a few rows
    # push one expert past 1024 slots (= 2 full PSUM chunks), keep the device
    # batch at 1024 and run the leftover rows on the host in fp32.
    loads = np.bincount(epos, minlength=E)
    max_load = int(loads.max())
    cap_k64 = max(64, -(-max_load // 64) * 64)
    overflow = int(np.maximum(loads - 1024, 0).sum())
    cap_k = 1024 if (cap_k64 > 1024 and overflow <= 192) \
        else min(CAP, cap_k64)

    on_dev = slot < cap_k
    idx_e = np.zeros((E, cap_k), np.int64)
    w_e = np.zeros((E, cap_k), np.float32)
    idx_e[epos[on_dev], slot[on_dev]] = token[on_dev]
    w_e[epos[on_dev], slot[on_dev]] = wgt[on_dev]

    # ---------------- launch B ----------------
    # fp8(e4m3) quantization: activations cast directly (|xln2| ~ 4.7, well
    # inside e4m3 normal range); weights scaled to ~224 absmax (shared per
    # mf-pair so one gelu op can drain a 2-bank PSUM region), dequant folded
    # into the PSUM-drain ops on device.
    xln2_q8 = np.clip(xln2, -240, 240).astype(E4)
    exp_fc_w = np.asarray(exp_fc_w, np.float32)
    exp_fc_b = np.asarray(exp_fc_b, np.float32).reshape(E, F)
    exp_proj_w = np.asarray(exp_proj_w, np.float32)
    exp_proj_b = np.asarray(exp_proj_b, np.float32).reshape(E, C)

    G = 4
    fcb_r = exp_fc_b.reshape(E, KSUB_F // G, G, P)
    paired = bool((fcb_r == fcb_r[:, :, :1]).all())

    in_maps_b = []
    for e in range(E):
        xbT = _kperm(np.ascontiguousarray(xln2_q8[idx_e[e]].T))
        a1 = np.abs(exp_fc_w[e]).max(0).reshape(KSUB_F // G, G, P)
        if paired:
            gmax = a1.max(1)                                  # [6, p]
            s1g = 224.0 / np.maximum(gmax, 1e-30)
            s1 = np.repeat(s1g, G, axis=0).reshape(F)
            fcb_h = np.ascontiguousarray(fcb_r[e, :, 0].T)
        else:
            s1g = 224.0 / np.maximum(a1.reshape(KSUB_F, P), 1e-30)
            s1 = s1g.reshape(F)
            fcb_h = np.ascontiguousarray(fcb_r[e].reshape(KSUB_F, P).T)
        s2 = 224.0 / np.maximum(np.abs(exp_proj_w[e]).max(0), 1e-30)  # [C]
        fcw = np.clip(exp_fc_w[e] * s1, -240, 240).astype(E4)
        fcw = fcw.reshape(CSUB, P, KSUB_F, P).transpose(1, 2, 0, 3)
        pjw = np.clip(exp_proj_w[e] * s2, -240, 240).astype(E4)
        pjw = pjw.reshape(KSUB_F, P, CSUB, P).transpose(1, 2, 0, 3)
        scb = np.concatenate([
            (1.0 / s1g).T, fcb_h,
            _pcol(1.0 / s2, CSUB), _pcol(exp_proj_b[e], CSUB)], axis=1)
        in_maps_b.append({
            "xbT": xbT,
            "fcw": np.ascontiguousarray(fcw.reshape(P, -1)),
            "pjw": np.ascontiguousarray(pjw.reshape(P, -1)),
            "scb": np.ascontiguousarray(scb.astype(np.float32)),
        })

    if ("expert", cap_k, paired) not in _CACHE:
        _CACHE[("expert", cap_k, paired)] = build_expert(cap_k, paired)
    res_b = _run_spmd(_CACHE[("expert", cap_k, paired)], in_maps_b)

    y = xf2.copy()
    for e in range(E):
        valid = w_e[e] != 0
        outT = np.asarray(res_b.results[e]["outT"]).astype(np.float32)
        y[idx_e[e, valid]] += w_e[e, valid, None] * outT.T[valid]

    # host top-up for the few rows beyond cap_k (exact fp32)
    if not on_dev.all():
        try:
            from scipy.special import erf
        except ImportError:
            erf = np.vectorize(math.erf)
        off = ~on_dev
        for e in np.unique(epos[off]):
            m = off & (epos == e)
            tk = token[m]
            h = xln2[tk] @ exp_fc_w[e] + exp_fc_b[e]
            h = 0.5 * h * (1.0 + erf(h / math.sqrt(2.0)))
            o = h @ exp_proj_w[e] + exp_proj_b[e]
            y[tk] += wgt[m, None] * o
    return y.reshape(B, T, C).astype(np.float32)



# revision 62
# speedup vs baseline: 1.0042x; 1.0042x over previous
"""MoE transformer block on 8 Trainium2 cores.

Layer: x = x + attn(ln1(x)); x = x + moe(ln2(x)).
Shapes: B=4, T=1024, C=768, H=12 heads, E=8 experts, top-2, cap=1280, F=3072.

Distribution:
  Launch A (attention, bf16): core i -> batch i//2, heads 6*(i%2) .. +6.
    LN1 (affine folded into the QKV weights) is applied host-side and x-hat
    arrives pre-transposed [C, T]. The head loop is software-pipelined depth
    2 (AV of head h trails scores of head h+2) so the softmax-exp stream on
    the Activation engine — the middle-phase critical path — never starves.
    Each core emits a partial (6-head) projection, transposed [C, T] bf16;
    host sums the two half-head partials per batch and adds the residual.
  Host: ln2 + gating + exact top-2 capacity routing (numpy, matches the jax
    reference in ordering; near-tie tokens get exact fp32 logits), builds
    per-expert gather indices.
  Launch B (experts, fp8): core e -> expert e, slots packed to min(observed
    max load rounded to 64, 1024). Both matmuls run fp8(e4m3) DoubleRow
    (K=256/instr, 0.5 cyc/row); weights are quantized per 4-mf group
    host-side, activations are cast directly (|x|<5 fits e4m3), dequant
    rides the PSUM-drain ops. Token dim is chunked 4x256 with mm2 pieces
    woven into the mm1/gelu stream. outT [C, cap_k] bf16; host scatter-adds
    w * out into y and computes overflow slots (beyond cap_k) in fp32.
"""

import math

import numpy as np
import ml_dtypes

import concourse.bacc as bacc
import concourse.bass as bass
import concourse.mybir as mybir
import concourse.tile as tile
from concourse import bass_utils
from concourse.masks import make_identity

F32 = mybir.dt.float32
BF16 = mybir.dt.bfloat16
FP8 = mybir.dt.float8e4
E4 = ml_dtypes.float8_e4m3  # matches TRN float8e4 (max ±240)
AF = mybir.ActivationFunctionType
ALU = mybir.AluOpType
AX = mybir.AxisListType

B, T, C = 4, 1024, 768
NHEAD = 12
HD = C // NHEAD  # 64
E = 8
TOPK = 2
CAP = 1280
F = 4 * C  # 3072
LN_EPS = 1e-5
NEG_INF = -1e30
P = 128

N_CORES = 8
H6 = NHEAD // 2          # heads per core
D6 = H6 * HD             # 384
CSUB = C // P            # 6
KSUB_F = F // P          # 24
NT = T // P              # 8
QKV9 = 3 * D6 // P       # 9

_CACHE = {}


def _chunks(n, step=512):
    out = []
    s = 0
    while s < n:
        out.append((s, min(step, n - s)))
        s += step
    return out


def _run_spmd(nc, in_maps):
    """run_bass_kernel_spmd with one retry (transient NRT/axon failures)."""
    try:
        return bass_utils.run_bass_kernel_spmd(
            nc, in_maps, core_ids=list(range(N_CORES)))
    except Exception:
        import time as _time
        _time.sleep(2.0)
        return bass_utils.run_bass_kernel_spmd(
            nc, in_maps, core_ids=list(range(N_CORES)))


# --------------------------------------------------------------------------
# Launch A: attention
# --------------------------------------------------------------------------

def build_attn():
    nc = bacc.Bacc("TRN2", target_bir_lowering=False, debug=False)

    # LN1 is applied host-side (host already computes the stats); the kernel
    # receives x-hat transposed [C, T] so the qkv matmuls start immediately.
    xlnTd = nc.dram_tensor("xlnT", [P, CSUB, T], BF16, kind="ExternalInput")
    # qkv weight slice for this core's 6 heads, ln1-folded, q pre-scaled by
    # 1/sqrt(HD), grouped (0,3,6),(1,4,7),(2,5,8) — the order the head loop
    # consumes the 128-col groups. column order within n:
    # q h0..h5 | k h0..h5 | v h0..h5 (64 cols each head)
    wqkv = nc.dram_tensor("wqkv", [P, QKV9 * CSUB * P], BF16,
                          kind="ExternalInput")
    wpj = nc.dram_tensor("wpj", [P, D6 // P, C], BF16, kind="ExternalInput")
    # transposed causal 0/1 mask (bf16): cmaskT[k, q] = 1 if k <= q else 0
    cmaskT = nc.dram_tensor("cmaskT", [P, P], BF16, kind="ExternalInput")
    # packed [bqkv(9) | bpj(6)] — one DMA; HWDGE setup is ~0.6us per copy,
    # so copy count gates the lead-in
    meta = nc.dram_tensor("meta", [P, QKV9 + CSUB], F32, kind="ExternalInput")
    out = nc.dram_tensor("attn_pT", [C, T], BF16, kind="ExternalOutput")

    with tile.TileContext(nc) as tc:
        with (
            tc.tile_pool(name="const", bufs=1) as const,
            tc.tile_pool(name="big", bufs=1) as big,
            tc.tile_pool(name="pTp", bufs=5) as pTp,
            tc.tile_pool(name="work", bufs=4) as work,
            tc.tile_pool(name="ps_sc", bufs=2, space="PSUM") as ps_sc,
            tc.tile_pool(name="ps_mm", bufs=1, space="PSUM") as ps_mm,
            tc.tile_pool(name="ps_t", bufs=1, space="PSUM") as ps_t,
            tc.tile_pool(name="ps_y", bufs=2, space="PSUM") as ps_y,
        ):
            # PE warmup during the DMA lead-in (p-state ramp)
            wz = const.tile([P, 512], BF16, name="wz")
            nc.vector.memset(wz[:], 0.0)
            for wi in range(12):
                pw = ps_mm.tile([P, 512], F32, tag="mm", name=f"warm{wi}")
                nc.tensor.matmul(pw[:], lhsT=wz[:, :P], rhs=wz[:],
                                 start=True, stop=True)

            meta_sb = const.tile([P, QKV9 + CSUB], F32)
            nc.sync.dma_start(meta_sb[:], meta[:])
            bqkv_sb = meta_sb[:, 0:QKV9]
            bpj_sb = meta_sb[:, QKV9:]
            cm = const.tile([P, P], BF16)
            nc.sync.dma_start(cm[:], cmaskT[:])
            xlnT = [big.tile([P, CSUB, T // 2], BF16, name=f"xlnT{i}")
                    for i in range(2)]
            wq_g = [const.tile([P, 3, CSUB, P], BF16, name=f"wqg{gi}")
                    for gi in range(3)]

            def dma_wq(gi):
                blk = 3 * CSUB * P
                nc.sync.dma_start(
                    wq_g[gi][:].rearrange("p a b c -> p (a b c)"),
                    wqkv[:, gi * blk:(gi + 1) * blk])

            nc.sync.dma_start(xlnT[0][:], xlnTd[:, :, 0:T // 2])
            dma_wq(0)
            nc.sync.dma_start(xlnT[1][:], xlnTd[:, :, T // 2:T])
            dma_wq(1)
            dma_wq(2)
            ident = const.tile([P, P], BF16)
            make_identity(nc, ident[:])
            wpj_sb = const.tile([P, D6 // P, C], BF16)
            nc.sync.dma_start(wpj_sb[:], wpj[:])

            # ---- qkvT [3*D6, T] = wqkv.T @ xln.T, + bias
            # one SBUF tile per 128-row group so consumers wait only on the
            # rows they read
            qkvT = [big.tile([P, T], BF16, tag=f"qkvT{mc}", name=f"qkvT{mc}")
                    for mc in range(QKV9)]
            v_ones = big.tile([P, NT, H6, 1 + 64], BF16)
            nc.vector.memset(v_ones[:, :, :, 0:1], 1.0)
            y_big = big.tile([P, NT, D6], BF16)

            def emit_qkv(mc, use_sc=False):
                for th in range(T // 512):
                    # before any scores exist the sc ring is idle — borrow
                    # it for every other early chunk so the mm ring's
                    # drain-read latency never bubbles the PE
                    if use_sc and th % 2 == 1:
                        pacc = ps_sc.tile([P, 512], F32, tag="sc",
                                          name=f"qk{mc}{th}")
                    else:
                        pacc = ps_mm.tile([P, 512], F32, tag="mm",
                                          name=f"qk{mc}{th}")
                    for ks in range(CSUB):
                        nc.tensor.matmul(
                            pacc[:, :512],
                            lhsT=wq_g[mc % 3][:, mc // 3, ks, :],
                            rhs=xlnT[th][:, ks, :],
                            start=(ks == 0), stop=(ks == CSUB - 1))
                    # bias+copy PSUM->SBUF (PSUM is DVE/ACT-only)
                    nc.vector.tensor_scalar_add(
                        qkvT[mc][:, th * 512:(th + 1) * 512], pacc[:, :512],
                        bqkv_sb[:, mc:mc + 1])

            def emit_vones(j):
                # vT row j -> v for heads 2j, 2j+1 (col 0 stays all-ones).
                # All 8 transposes land in one PSUM tile, drained by a single
                # Pool copy, so the PE rips through without ring round-trips.
                pt8 = ps_t.tile([P, NT, P], BF16, tag="pt8", name=f"vt{j}")
                for ti in range(NT):
                    nc.tensor.transpose(
                        pt8[:, ti, :],
                        qkvT[2 * (D6 // P) + j][:, ti * P:(ti + 1) * P],
                        ident[:])
                nc.vector.tensor_copy(
                    v_ones[:, :, 2 * j:2 * j + 2, 1:],
                    pt8[:].rearrange("p t (a b) -> p t a b", a=2))

            def emit_scores(h, pT):
                # scores transposed sT[k, q] so Exp lands pT in SBUF directly;
                # the causal mask of the diagonal block is added by the PE.
                qp0 = 64 * (h % 2)
                qrow = h // 2
                kp0 = (D6 + 64 * h) % P
                krow = (D6 + 64 * h) // P
                for kb in range(NT):
                    q0 = kb * P
                    pscore = ps_sc.tile([P, T], F32, tag="sc",
                                        name=f"sc{h}{kb}")
                    # chunk on absolute 512 boundaries (PSUM bank alignment
                    # for the matmuls); Exp drains the whole row in one op
                    bounds = [q0] + [b for b in (512, T) if b > q0]
                    for (s0, e0) in zip(bounds[:-1], bounds[1:]):
                        w = e0 - s0
                        nc.tensor.matmul(
                            pscore[:, s0:s0 + w],
                            lhsT=qkvT[krow][kp0:kp0 + 64, kb * P:(kb + 1) * P],
                            rhs=qkvT[qrow][qp0:qp0 + 64, s0:s0 + w],
                            start=True, stop=True)
                    nc.scalar.activation(
                        pT[:, kb, q0:], pscore[:, q0:], AF.Exp)
                    # causal mask of the diagonal block: 0/1 multiply on the
                    # near-idle Pool engine (SBUF-only op) instead of a PE
                    # matmul add — exp of the unmasked scores stays finite
                    # in bf16 (|s| < ~20)
                    nc.gpsimd.tensor_tensor(
                        pT[:, kb, q0:q0 + P], pT[:, kb, q0:q0 + P],
                        cm[:], op=ALU.mult)

            yT = [big.tile([P, D6 // P, T // 2], BF16, tag=f"yT{i}",
                           name=f"yT{i}") for i in range(2)]

            def emit_av(h, pT):
                # AV for head h, softmax denominator fused via v_ones col 0;
                # normalization batched per qi-pair on DVE
                for q2 in range(NT // 2):
                    py = ps_y.tile([P, 2, 65], F32, tag="py",
                                   name=f"py{h}{q2}")
                    for qq in range(2):
                        qi = 2 * q2 + qq
                        for kb in range(qi + 1):
                            nc.tensor.matmul(
                                py[:, qq, :],
                                lhsT=pT[:, kb, qi * P:(qi + 1) * P],
                                rhs=v_ones[:, kb, h, :],
                                start=(kb == 0), stop=(kb == qi))
                    rec = work.tile([P, 2], F32, tag="rec")
                    nc.vector.reciprocal(rec[:], py[:, :, 0:1])
                    nc.vector.tensor_tensor(
                        y_big[:, 2 * q2:2 * q2 + 2, h * 64:(h + 1) * 64],
                        py[:, :, 1:],
                        rec[:].to_broadcast([P, 2, 64]), op=ALU.mult)

            def emit_yt(g):
                # batched yT transpose for the 128-row block of heads 2g/2g+1
                pt8 = ps_t.tile([P, NT, P], BF16, tag="pt8", name=f"yt{g}")
                for qi in range(NT):
                    nc.tensor.transpose(
                        pt8[:, qi, :], y_big[:, qi, g * P:(g + 1) * P],
                        ident[:])
                for i in range(2):
                    nc.vector.tensor_copy(
                        yT[i][:, g, :], pt8[:, 4 * i:4 * i + 4, :]
                        .rearrange("p a b -> p (a b)"))

            # head-level software pipelining, depth 2: AV(h) trails
            # scores(h+2), so the exp stream on ACT (the middle-phase
            # critical path) never starves while the in-order PE waits on
            # exp-gated AV work
            pTs = []

            def emit_block(h):
                g = h // 2
                if h % 2 == 0:
                    emit_qkv(g, use_sc=(h == 0))
                    emit_qkv(3 + g, use_sc=(h == 0))
                else:
                    emit_qkv(6 + g)
                    emit_vones(g)
                pT = pTp.tile([P, NT, T], BF16, tag="pT", name=f"pT{h}")
                emit_scores(h, pT)
                pTs.append(pT)

            emit_block(0)
            emit_block(1)
            emit_block(2)
            for h in range(H6):
                if h + 3 < H6:
                    emit_block(h + 3)
                emit_av(h, pTs[h])
                if h % 2 == 1:
                    emit_yt(h // 2)

            # ---- partial projection: outT [C, T] = wpj.T @ y.T + bpj
            # pacc alternates between the two PSUM rings so chunks pipeline;
            # drains alternate DVE/ACT (exp stream is over; Identity shares
            # every act table so no reload); the last c-tile DMAs per half
            # so the final transfer is small
            for cc in range(CSUB):
                o_sb = work.tile([P, T], BF16, tag="osb")
                for th in range(T // 512):
                    pool = ps_mm if (2 * cc + th) % 2 == 0 else ps_sc
                    tag = "mm" if pool is ps_mm else "sc"
                    pacc = pool.tile([P, 512], F32, tag=tag,
                                     name=f"pj{cc}{th}")
                    for j in range(D6 // P):
                        nc.tensor.matmul(
                            pacc[:, :512],
                            lhsT=wpj_sb[:, j, cc * P:(cc + 1) * P],
                            rhs=yT[th][:, j, :],
                            start=(j == 0), stop=(j == D6 // P - 1))
                    if th == 0:
                        nc.vector.tensor_scalar_add(
                            o_sb[:, :512], pacc[:, :512],
                            bpj_sb[:, cc:cc + 1])
                    else:
                        nc.scalar.activation(
                            o_sb[:, 512:], pacc[:, :512], AF.Identity,
                            bias=bpj_sb[:, cc:cc + 1])
                    if cc == CSUB - 1:
                        nc.sync.dma_start(
                            out[cc * P:(cc + 1) * P,
                                th * 512:(th + 1) * 512],
                            o_sb[:, th * 512:(th + 1) * 512])
                if cc < CSUB - 1:
                    nc.sync.dma_start(out[cc * P:(cc + 1) * P, :], o_sb[:])

    nc.compile()
    return nc


# --------------------------------------------------------------------------
# Launch B: experts
# --------------------------------------------------------------------------

def build_expert(cap_k, grouped=True):
    """fp8(e4m3) expert MLP. Both matmuls run in DoubleRow perf mode (K=256
    per instruction, 0.5 cyc/row). Weights are quantized host-side with
    scales shared per 4-mf group; dequant rides the PSUM-drain op. hT stays
    fp8 in SBUF (mm2's rhs must be fp8). Token dim is processed in 256-col
    chunks, interleaving mm1/mm2 chunks on the (in-order) PE so mm2 work
    hides behind the gelu stream on ACT — which is the pacing engine.
    `grouped` (one ACT op per 4-mf [P,4,256] PSUM region) requires
    group-equal biases; fallback is one gelu per mf."""
    nc = bacc.Bacc("TRN2", target_bir_lowering=False, debug=False)

    G = 4                    # mf group per gelu op
    NQ = KSUB_F // G         # 6 groups
    xbT = nc.dram_tensor("xbT", [P, CSUB, cap_k], FP8, kind="ExternalInput")
    fcw = nc.dram_tensor("fcw", [P, KSUB_F * CSUB * P], FP8,
                         kind="ExternalInput")
    nsc = NQ if grouped else KSUB_F
    # packed [fcs | fcb | pjs | pjb] — one DMA
    scb = nc.dram_tensor("scb", [P, 2 * nsc + 2 * CSUB], F32,
                         kind="ExternalInput")
    pjw = nc.dram_tensor("pjw", [P, CSUB * KSUB_F * P], FP8,
                         kind="ExternalInput")
    out = nc.dram_tensor("outT", [C, cap_k], BF16, kind="ExternalOutput")

    SC = _chunks(cap_k, 256)          # compute chunks
    SD = _chunks(cap_k, 512)          # xbT DMA pieces (512B runs)
    DR = mybir.MatmulPerfMode.DoubleRow
    MFBLK = CSUB * P                  # 768 fp8 bytes per mf per partition

    NCH = len(SC)
    with tile.TileContext(nc) as tc:
        with (
            tc.tile_pool(name="const", bufs=1) as const,
            tc.tile_pool(name="osb", bufs=4) as osbp,
            tc.tile_pool(name="ps1", bufs=2, space="PSUM") as ps1,
            tc.tile_pool(name="ps2", bufs=3, space="PSUM") as ps2,
            tc.tile_pool(name="psw", bufs=1, space="PSUM") as psw,
        ):
            # PE warmup during the xbT/weight DMA lead-in
            wz = const.tile([P, 512], BF16, name="wz")
            nc.vector.memset(wz[:], 0.0)
            for wi in range(6):
                pw = psw.tile([P, 512], F32, tag="warm", name=f"warm{wi}")
                nc.tensor.matmul(pw[:], lhsT=wz[:, :P], rhs=wz[:],
                                 start=True, stop=True)

            # NOTE: tile-granular dependency tracking — every dma/compute
            # producer gets its own tile so consumers wait only on what they
            # actually read. One packed tile for the four tiny scale/bias
            # vectors (single DMA; HWDGE setup is ~0.6us per copy).
            sc_sb = const.tile([P, 2 * nsc + 2 * CSUB], F32)
            fcs_sb = sc_sb[:, :nsc]
            fcb_sb = sc_sb[:, nsc:2 * nsc]
            pjs_sb = sc_sb[:, 2 * nsc:2 * nsc + CSUB]
            pjb_sb = sc_sb[:, 2 * nsc + CSUB:]

            xb_t = [const.tile([P, CSUB, sw], FP8, name=f"xb{i}")
                    for i, (s0, sw) in enumerate(SD)]
            # quad 0 split in two 2-mf tiles: its DMA gates the very first
            # matmul, so halving the first transfer shaves the lead-in
            w1_q0 = [const.tile([P, 2, CSUB, P], FP8, name=f"w1q0{h}")
                     for h in range(2)]
            w1_t = [const.tile([P, G, CSUB, P], FP8, name=f"w1q{q}")
                    for q in range(1, NQ)]

            def w1_lhsT(q, g):
                if q == 0:
                    return w1_q0[g // 2][:, g % 2]
                return w1_t[q - 1][:, g]
            w2_t = [const.tile([P, KSUB_F, P], FP8, name=f"w2c{cc}")
                    for cc in range(CSUB)]
            hT_t = [const.tile([P, KSUB_F, sw], FP8, name=f"hT{i}")
                    for i, (s0, sw) in enumerate(SC)]

            def dma_xbT(i):
                s0, sw = SD[i]
                nc.sync.dma_start(xb_t[i][:], xbT[:, :, s0:s0 + sw])

            def dma_w1(q):
                if q == 0:
                    for h in range(2):
                        nc.sync.dma_start(
                            w1_q0[h][:].rearrange("p a b c -> p (a b c)"),
                            fcw[:, 2 * h * MFBLK:2 * (h + 1) * MFBLK])
                    return
                nc.sync.dma_start(
                    w1_t[q - 1][:].rearrange("p a b c -> p (a b c)"),
                    fcw[:, q * G * MFBLK:(q + 1) * G * MFBLK])

            def dma_w2(cc):
                blk = KSUB_F * P
                nc.sync.dma_start(
                    w2_t[cc][:].rearrange("p a b -> p (a b)"),
                    pjw[:, cc * blk:(cc + 1) * blk])

            # ordered by first consumer: xbT piece 0 and all w1 first (they
            # pace mm1(c0) and with it the whole gelu stream), then w2
            # staged around the late-needed xbT tail
            dma_xbT(0)
            dma_w1(0)
            nc.sync.dma_start(sc_sb[:], scb[:])
            for q in range(1, NQ):
                dma_w1(q)
            dma_w2(0)
            dma_w2(1)
            dma_w2(2)
            for i in range(1, len(SD)):
                dma_xbT(i)
            dma_w2(3)
            dma_w2(4)
            dma_w2(5)

            def piece_of(s0):
                for i, (p0, pw) in enumerate(SD):
                    if p0 <= s0 < p0 + pw:
                        return i, s0 - p0
                raise AssertionError(s0)

            def mm1(ci):
                s0, sw = SC[ci]
                di, d0 = piece_of(s0)
                for q in range(NQ):
                    if ci == 0:
                        # filler: the first chunk is paced by the w1 quad
                        # DMAs; keep the PE p-state warm across the ~0.4us
                        # per-quad wait
                        pw = psw.tile([P, 512], F32, tag="warm",
                                      name=f"fill{q}")
                        nc.tensor.matmul(pw[:], lhsT=wz[:, :P], rhs=wz[:],
                                         start=True, stop=True)
                    pacc = ps1.tile([P, G, 256], F32, tag="mm1")
                    for g in range(G):
                        for j in range(CSUB // 2):
                            nc.tensor.matmul(
                                pacc[:, g, :sw],
                                lhsT=w1_lhsT(q, g)[:, 2 * j:2 * j + 2, :],
                                rhs=xb_t[di][:, 2 * j:2 * j + 2, d0:d0 + sw],
                                start=(j == 0), stop=(j == CSUB // 2 - 1),
                                perf_mode=DR)
                    if grouped:
                        nc.scalar.activation(
                            hT_t[ci][:, G * q:G * q + G, :sw],
                            pacc[:, :, :sw], AF.Gelu,
                            bias=fcb_sb[:, q:q + 1],
                            scale=fcs_sb[:, q:q + 1])
                    else:
                        for g in range(G):
                            mf = G * q + g
                            nc.scalar.activation(
                                hT_t[ci][:, mf, :sw],
                                pacc[:, g, :sw], AF.Gelu,
                                bias=fcb_sb[:, mf:mf + 1],
                                scale=fcs_sb[:, mf:mf + 1])

            def mm2(ci, cc):
                # one (chunk, output-c-tile) piece: parks the in-order PE
                # only on its own w2 tile / hT chunk
                s0, sw = SC[ci]
                pacc = ps2.tile([P, 256], F32, tag="mm2")
                for j in range(KSUB_F // 2):
                    nc.tensor.matmul(
                        pacc[:, :sw],
                        lhsT=w2_t[cc][:, 2 * j:2 * j + 2, :],
                        rhs=hT_t[ci][:, 2 * j:2 * j + 2, :sw],
                        start=(j == 0), stop=(j == KSUB_F // 2 - 1),
                        perf_mode=DR)
                # dequant+bias on DVE; ACT is saturated by mm1's gelu
                o_sb = osbp.tile([P, 256], BF16, tag="osb")
                nc.vector.tensor_scalar(
                    o_sb[:, :sw], pacc[:, :sw],
                    pjs_sb[:, cc:cc + 1], pjb_sb[:, cc:cc + 1],
                    op0=ALU.mult, op1=ALU.add)
                nc.sync.dma_start(
                    out[cc * P:(cc + 1) * P, s0:s0 + sw], o_sb[:, :sw])

            # PE program order: the gelu stream on ACT (1.09us per quad-op)
            # paces the pipeline; mm1(c) quads are throttled to it by the
            # ps1 rotation, so weave the previous chunk's mm2 pieces into
            # the stream to keep the in-order PE from idling
            mm1(0)
            for i in range(1, NCH):
                mm1(i)
                for cc in range(CSUB):
                    mm2(i - 1, cc)
            for cc in range(CSUB):
                mm2(NCH - 1, cc)

    nc.compile()
    return nc


# --------------------------------------------------------------------------
# Host glue
# --------------------------------------------------------------------------

def _bf16(a):
    return np.asarray(a, np.float32).astype(ml_dtypes.bfloat16)


def _pcol(vec, nsub):
    """[nsub*P] -> [P, nsub] per-partition bias layout."""
    return np.ascontiguousarray(
        np.asarray(vec, np.float32).reshape(nsub, P).T)


def _kperm(w):
    """[K, N] -> [P, K//P, N] partition-major layout, contiguous."""
    k, n = w.shape
    return np.ascontiguousarray(w.reshape(k // P, P, n).transpose(1, 0, 2))


def _layer_norm(x, w, b):
    mu = x.mean(-1, keepdims=True)
    var = x.var(-1, keepdims=True)
    return (x - mu) / np.sqrt(var + LN_EPS) * w + b


def _exact_logits(need, x, ln1_w, ln1_b, ln2_w, ln2_b, qkv_w, qkv_b,
                  proj_w, proj_b, w_g):
    """fp32 gating logits for the given flat token indices (exact attention
    rows for just those tokens)."""
    out = np.empty((need.size, E), np.float32)
    bs, ps = need // T, need % T
    for b in np.unique(bs):
        m = bs == b
        pos = ps[m]                              # [M]
        xl = _layer_norm(x[b], ln1_w, ln1_b)     # [T, C]
        kv = xl @ qkv_w[:, C:] + qkv_b[C:]       # [T, 2C]
        k = kv[:, :C].reshape(T, NHEAD, HD)
        v = kv[:, C:].reshape(T, NHEAD, HD)
        q = (xl[pos] @ qkv_w[:, :C] + qkv_b[:C]).reshape(-1, NHEAD, HD)
        s = np.einsum("mhd,khd->mhk", q, k) / math.sqrt(HD)
        s = np.where(pos[:, None, None] >= np.arange(T)[None, None, :],
                     s, NEG_INF)
        s -= s.max(-1, keepdims=True)
        p = np.exp(s)
        p /= p.sum(-1, keepdims=True)
        y = np.einsum("mhk,khd->mhd", p, v).reshape(-1, C)
        att = y @ proj_w + proj_b
        x2 = x[b][pos] + att
        out[m] = _layer_norm(x2, ln2_w, ln2_b) @ w_g
    return out


def kernel(x, ln1_w, ln1_b, ln2_w, ln2_b, attn_qkv_w, attn_qkv_b,
           attn_proj_w, attn_proj_b, w_g, exp_fc_w, exp_fc_b,
           exp_proj_w, exp_proj_b):
    x = np.asarray(x, np.float32)
    ln1_w = np.asarray(ln1_w, np.float32)
    ln1_b = np.asarray(ln1_b, np.float32)
    attn_qkv_w = np.asarray(attn_qkv_w, np.float32)
    attn_qkv_b = np.asarray(attn_qkv_b, np.float32)
    attn_proj_w = np.asarray(attn_proj_w, np.float32)
    attn_proj_b = np.asarray(attn_proj_b, np.float32)

    if "attn" not in _CACHE:
        _CACHE["attn"] = build_attn()

    # ---------------- launch A ----------------
    # fold ln1 affine into qkv: qkv = xhat @ (diag(w1) W) + (b1 @ W + b)
    Wf = ln1_w[:, None] * attn_qkv_w          # [C, 3C]
    bf = ln1_b @ attn_qkv_w + attn_qkv_b      # [3C]
    Wq = Wf[:, :C] / math.sqrt(HD)
    bq = bf[:C] / math.sqrt(HD)
    Wk, bk = Wf[:, C:2 * C], bf[C:2 * C]
    Wv, bv = Wf[:, 2 * C:], bf[2 * C:]

    cmaskT_np = _bf16(np.triu(np.ones((P, P), np.float32)))

    in_maps_a = []
    for core in range(N_CORES):
        b = core // 2
        h0 = H6 * (core % 2)
        cols = slice(h0 * HD, (h0 + H6) * HD)
        wqkv_c = np.concatenate([Wq[:, cols], Wk[:, cols], Wv[:, cols]], 1)
        bqkv_c = np.concatenate([bq[cols], bk[cols], bv[cols]])
        bpj_c = attn_proj_b if core % 2 == 0 else np.zeros(C, np.float32)
        mu_b = x[b].mean(-1)
        rstd_b = 1.0 / np.sqrt(x[b].var(-1) + LN_EPS)
        xln_b = (x[b] - mu_b[:, None]) * rstd_b[:, None]    # [T, C]
        xlnT_h = _bf16(xln_b).T.reshape(CSUB, P, T).transpose(1, 0, 2)
        wqkv_m = _bf16(wqkv_c).reshape(CSUB, P, QKV9, P)
        wqkv_m = wqkv_m.transpose(1, 2, 0, 3)[:, [0, 3, 6, 1, 4, 7, 2, 5, 8]]
        meta = np.concatenate([
            _pcol(bqkv_c, QKV9), _pcol(bpj_c, CSUB)], axis=1)
        in_maps_a.append({
            "xlnT": np.ascontiguousarray(xlnT_h),
            "meta": np.ascontiguousarray(meta.astype(np.float32)),
            "wqkv": np.ascontiguousarray(wqkv_m.reshape(P, -1)),
            "wpj": _kperm(_bf16(attn_proj_w[h0 * HD:(h0 + H6) * HD, :])),
            "cmaskT": cmaskT_np,
        })

    res_a = _run_spmd(_CACHE["attn"], in_maps_a)

    attn = np.empty((B, T, C), np.float32)
    for b in range(B):
        attn[b] = (np.asarray(res_a.results[2 * b]["attn_pT"], np.float32)
                   + np.asarray(res_a.results[2 * b + 1]["attn_pT"],
                                np.float32)).T

    x2 = x + attn                       # [B, T, C]
    xf2 = x2.reshape(B * T, C)

    # ---------------- host routing (exact reference semantics) -------------
    N = B * T
    xln2 = _layer_norm(xf2, np.asarray(ln2_w, np.float32),
                       np.asarray(ln2_b, np.float32))
    logits = xln2 @ np.asarray(w_g, np.float32)        # [N, E]

    # The top-2 expert choice is discontinuous: tokens whose top2/top3 gating
    # logits are within the bf16 noise floor could route differently than the
    # fp32 reference would. Recompute those few tokens' logits exactly.
    srt = np.sort(logits, axis=1)
    need = np.nonzero(srt[:, -2] - srt[:, -3] < 0.02)[0]
    if need.size:
        logits[need] = _exact_logits(
            need, x, ln1_w, ln1_b, np.asarray(ln2_w, np.float32),
            np.asarray(ln2_b, np.float32), attn_qkv_w, attn_qkv_b,
            attn_proj_w, attn_proj_b, np.asarray(w_g, np.float32))

    order = np.argsort(-logits, axis=1, kind="stable")
    topk_idx = order[:, :TOPK]                          # [N, K]
    sel = np.zeros((N, E), bool)
    np.put_along_axis(sel, topk_idx, True, axis=1)
    masked = np.where(sel, logits, NEG_INF)
    m = masked.max(1, keepdims=True)
    ex = np.exp(masked - m)
    router_probs = ex / ex.sum(1, keepdims=True)        # [N, E]

    # capacity ranks in (k, n) order
    exp_mask = np.zeros((TOPK, N, E), np.int64)
    kk = np.arange(TOPK)[:, None]
    nn = np.arange(N)[None, :]
    exp_mask[kk, nn, topk_idx.T] = 1
    flat = exp_mask.reshape(TOPK * N, E)
    rank = np.cumsum(flat, axis=0) - 1                  # [K*N, E]
    keep = (flat == 1) & (rank < CAP)
    kpos, epos = np.nonzero(keep)
    token = kpos % N
    slot = rank[kpos, epos]
    wgt = router_probs[token, epos]

    # pack the expert batches to the observed max load; if only a few rows
    # push one expert past 1024 slots (= 2 full PSUM chunks), keep the device
    # batch at 1024 and run the leftover rows on the host in fp32.
    loads = np.bincount(epos, minlength=E)
    max_load = int(loads.max())
    cap_k64 = max(64, -(-max_load // 64) * 64)
    overflow = int(np.maximum(loads - 1024, 0).sum())
    cap_k = 1024 if (cap_k64 > 1024 and overflow <= 192) \
        else min(CAP, cap_k64)

    on_dev = slot < cap_k
    idx_e = np.zeros((E, cap_k), np.int64)
    w_e = np.zeros((E, cap_k), np.float32)
    idx_e[epos[on_dev], slot[on_dev]] = token[on_dev]
    w_e[epos[on_dev], slot[on_dev]] = wgt[on_dev]

    # ---------------- launch B ----------------
    # fp8(e4m3) quantization: activations cast directly (|xln2| ~ 4.7, well
    # inside e4m3 normal range); weights scaled to ~224 absmax (shared per
    # mf-pair so one gelu op can drain a 2-bank PSUM region), dequant folded
    # into the PSUM-drain ops on device.
    xln2_q8 = np.clip(xln2, -240, 240).astype(E4)
    exp_fc_w = np.asarray(exp_fc_w, np.float32)
    exp_fc_b = np.asarray(exp_fc_b, np.float32).reshape(E, F)
    exp_proj_w = np.asarray(exp_proj_w, np.float32)
    exp_proj_b = np.asarray(exp_proj_b, np.float32).reshape(E, C)

    G = 4
    fcb_r = exp_fc_b.reshape(E, KSUB_F // G, G, P)
    paired = bool((fcb_r == fcb_r[:, :, :1]).all())

    in_maps_b = []
    for e in range(E):
        xbT = _kperm(np.ascontiguousarray(xln2_q8[idx_e[e]].T))
        a1 = np.abs(exp_fc_w[e]).max(0).reshape(KSUB_F // G, G, P)
        if paired:
            gmax = a1.max(1)                                  # [6, p]
            s1g = 224.0 / np.maximum(gmax, 1e-30)
            s1 = np.repeat(s1g, G, axis=0).reshape(F)
            fcb_h = np.ascontiguousarray(fcb_r[e, :, 0].T)
        else:
            s1g = 224.0 / np.maximum(a1.reshape(KSUB_F, P), 1e-30)
            s1 = s1g.reshape(F)
            fcb_h = np.ascontiguousarray(fcb_r[e].reshape(KSUB_F, P).T)
        s2 = 224.0 / np.maximum(np.abs(exp_proj_w[e]).max(0), 1e-30)  # [C]
        fcw = np.clip(exp_fc_w[e] * s1, -240, 240).astype(E4)
        fcw = fcw.reshape(CSUB, P, KSUB_F, P).transpose(1, 2, 0, 3)
        pjw = np.clip(exp_proj_w[e] * s2, -240, 240).astype(E4)
        pjw = pjw.reshape(KSUB_F, P, CSUB, P).transpose(1, 2, 0, 3)
        scb = np.concatenate([
            (1.0 / s1g).T, fcb_h,
            _pcol(1.0 / s2, CSUB), _pcol(exp_proj_b[e], CSUB)], axis=1)
        in_maps_b.append({
            "xbT": xbT,
            "fcw": np.ascontiguousarray(fcw.reshape(P, -1)),
            "pjw": np.ascontiguousarray(pjw.reshape(P, -1)),
            "scb": np.ascontiguousarray(scb.astype(np.float32)),
        })

    if ("expert", cap_k, paired) not in _CACHE:
        _CACHE[("expert", cap_k, paired)] = build_expert(cap_k, paired)
    res_b = _run_spmd(_CACHE[("expert", cap_k, paired)], in_maps_b)

    y = xf2.copy()
    for e in range(E):
        valid = w_e[e] != 0
        outT = np.asarray(res_b.results[e]["outT"]).astype(np.float32)
        y[idx_e[e, valid]] += w_e[e, valid, None] * outT.T[valid]

    # host top-up for the few rows beyond cap_k (exact fp32)
    if not on_dev.all():
        try:
            from scipy.special import erf
        except ImportError:
            erf = np.vectorize(math.erf)
        off = ~on_dev
        for e in np.unique(epos[off]):
            m = off & (epos == e)
            tk = token[m]
            h = xln2[tk] @ exp_fc_w[e] + exp_fc_b[e]
            h = 0.5 * h * (1.0 + erf(h / math.sqrt(2.0)))
            o = h @ exp_proj_w[e] + exp_proj_b[e]
            y[tk] += wgt[m, None] * o
    return y.reshape(B, T, C).astype(np.float32)



# revision 64
# speedup vs baseline: 1.0126x; 1.0084x over previous
"""MoE transformer block on 8 Trainium2 cores.

Layer: x = x + attn(ln1(x)); x = x + moe(ln2(x)).
Shapes: B=4, T=1024, C=768, H=12 heads, E=8 experts, top-2, cap=1280, F=3072.

Distribution:
  Launch A (attention, bf16): core i -> batch i//2, heads 6*(i%2) .. +6.
    LN1 (affine folded into the QKV weights) is applied host-side and x-hat
    arrives pre-transposed [C, T]. The head loop is software-pipelined depth
    2 (AV of head h trails scores of head h+2) so the softmax-exp stream on
    the Activation engine — the middle-phase critical path — never starves.
    Each core emits a partial (6-head) projection, transposed [C, T] bf16;
    host sums the two half-head partials per batch and adds the residual.
  Host: ln2 + gating + exact top-2 capacity routing (numpy, matches the jax
    reference in ordering; near-tie tokens get exact fp32 logits), builds
    per-expert gather indices.
  Launch B (experts, fp8): core e -> expert e, slots packed to min(observed
    max load rounded to 64, 1024). Both matmuls run fp8(e4m3) DoubleRow
    (K=256/instr, 0.5 cyc/row); weights are quantized per 4-mf group
    host-side, activations are cast directly (|x|<5 fits e4m3), dequant
    rides the PSUM-drain ops. Token dim is chunked 4x256 with mm2 pieces
    woven into the mm1/gelu stream. outT [C, cap_k] bf16; host scatter-adds
    w * out into y and computes overflow slots (beyond cap_k) in fp32.
"""

import math

import numpy as np
import ml_dtypes

import concourse.bacc as bacc
import concourse.bass as bass
import concourse.mybir as mybir
import concourse.tile as tile
from concourse import bass_utils
from concourse.masks import make_identity

F32 = mybir.dt.float32
BF16 = mybir.dt.bfloat16
FP8 = mybir.dt.float8e4
E4 = ml_dtypes.float8_e4m3  # matches TRN float8e4 (max ±240)
AF = mybir.ActivationFunctionType
ALU = mybir.AluOpType
AX = mybir.AxisListType

B, T, C = 4, 1024, 768
NHEAD = 12
HD = C // NHEAD  # 64
E = 8
TOPK = 2
CAP = 1280
F = 4 * C  # 3072
LN_EPS = 1e-5
NEG_INF = -1e30
P = 128

N_CORES = 8
H6 = NHEAD // 2          # heads per core
D6 = H6 * HD             # 384
CSUB = C // P            # 6
KSUB_F = F // P          # 24
NT = T // P              # 8
QKV9 = 3 * D6 // P       # 9

_CACHE = {}


def _chunks(n, step=512):
    out = []
    s = 0
    while s < n:
        out.append((s, min(step, n - s)))
        s += step
    return out


def _run_spmd(nc, in_maps):
    """run_bass_kernel_spmd with one retry (transient NRT/axon failures)."""
    try:
        return bass_utils.run_bass_kernel_spmd(
            nc, in_maps, core_ids=list(range(N_CORES)))
    except Exception:
        import time as _time
        _time.sleep(2.0)
        return bass_utils.run_bass_kernel_spmd(
            nc, in_maps, core_ids=list(range(N_CORES)))


# --------------------------------------------------------------------------
# Launch A: attention
# --------------------------------------------------------------------------

def build_attn():
    nc = bacc.Bacc("TRN2", target_bir_lowering=False, debug=False)

    # LN1 is applied host-side (host already computes the stats); the kernel
    # receives x-hat transposed [C, T] so the qkv matmuls start immediately.
    xlnTd = nc.dram_tensor("xlnT", [P, CSUB, T], BF16, kind="ExternalInput")
    # qkv weight slice for this core's 6 heads, ln1-folded, q pre-scaled by
    # 1/sqrt(HD), grouped (0,3,6),(1,4,7),(2,5,8) — the order the head loop
    # consumes the 128-col groups. column order within n:
    # q h0..h5 | k h0..h5 | v h0..h5 (64 cols each head)
    wqkv = nc.dram_tensor("wqkv", [P, QKV9 * CSUB * P], BF16,
                          kind="ExternalInput")
    wpj = nc.dram_tensor("wpj", [P, D6 // P, C], BF16, kind="ExternalInput")
    # transposed causal 0/1 mask (bf16): cmaskT[k, q] = 1 if k <= q else 0
    cmaskT = nc.dram_tensor("cmaskT", [P, P], BF16, kind="ExternalInput")
    # packed [bqkv(9) | bpj(6)] — one DMA; HWDGE setup is ~0.6us per copy,
    # so copy count gates the lead-in
    meta = nc.dram_tensor("meta", [P, QKV9 + CSUB], F32, kind="ExternalInput")
    out = nc.dram_tensor("attn_pT", [C, T], BF16, kind="ExternalOutput")

    with tile.TileContext(nc) as tc:
        with (
            tc.tile_pool(name="const", bufs=1) as const,
            tc.tile_pool(name="big", bufs=1) as big,
            tc.tile_pool(name="pTp", bufs=5) as pTp,
            tc.tile_pool(name="work", bufs=4) as work,
            tc.tile_pool(name="ps_sc", bufs=2, space="PSUM") as ps_sc,
            tc.tile_pool(name="ps_mm", bufs=1, space="PSUM") as ps_mm,
            tc.tile_pool(name="ps_t", bufs=1, space="PSUM") as ps_t,
            tc.tile_pool(name="ps_y", bufs=2, space="PSUM") as ps_y,
        ):
            # PE warmup during the DMA lead-in (p-state ramp)
            wz = const.tile([P, 512], BF16, name="wz")
            nc.vector.memset(wz[:], 0.0)
            for wi in range(12):
                pw = ps_mm.tile([P, 512], F32, tag="mm", name=f"warm{wi}")
                nc.tensor.matmul(pw[:], lhsT=wz[:, :P], rhs=wz[:],
                                 start=True, stop=True)

            meta_sb = const.tile([P, QKV9 + CSUB], F32)
            nc.sync.dma_start(meta_sb[:], meta[:])
            bqkv_sb = meta_sb[:, 0:QKV9]
            bpj_sb = meta_sb[:, QKV9:]
            cm = const.tile([P, P], BF16)
            nc.sync.dma_start(cm[:], cmaskT[:])
            xlnT = [big.tile([P, CSUB, T // 2], BF16, name=f"xlnT{i}")
                    for i in range(2)]
            wq_g = [const.tile([P, 3, CSUB, P], BF16, name=f"wqg{gi}")
                    for gi in range(3)]

            def dma_wq(gi):
                blk = 3 * CSUB * P
                nc.sync.dma_start(
                    wq_g[gi][:].rearrange("p a b c -> p (a b c)"),
                    wqkv[:, gi * blk:(gi + 1) * blk])

            nc.sync.dma_start(xlnT[0][:], xlnTd[:, :, 0:T // 2])
            dma_wq(0)
            nc.sync.dma_start(xlnT[1][:], xlnTd[:, :, T // 2:T])
            dma_wq(1)
            dma_wq(2)
            ident = const.tile([P, P], BF16)
            make_identity(nc, ident[:])
            wpj_sb = const.tile([P, D6 // P, C], BF16)
            nc.sync.dma_start(wpj_sb[:], wpj[:])

            # ---- qkvT [3*D6, T] = wqkv.T @ xln.T, + bias
            # one SBUF tile per 128-row group so consumers wait only on the
            # rows they read
            qkvT = [big.tile([P, T], BF16, tag=f"qkvT{mc}", name=f"qkvT{mc}")
                    for mc in range(QKV9)]
            v_ones = big.tile([P, NT, H6, 1 + 64], BF16)
            nc.vector.memset(v_ones[:, :, :, 0:1], 1.0)
            y_big = big.tile([P, NT, D6], BF16)

            def emit_qkv(mc, use_sc=False):
                for th in range(T // 512):
                    # before any scores exist the sc ring is idle — borrow
                    # it for every other early chunk so the mm ring's
                    # drain-read latency never bubbles the PE
                    if use_sc and th % 2 == 1:
                        pacc = ps_sc.tile([P, 512], F32, tag="sc",
                                          name=f"qk{mc}{th}")
                    else:
                        pacc = ps_mm.tile([P, 512], F32, tag="mm",
                                          name=f"qk{mc}{th}")
                    for ks in range(CSUB):
                        nc.tensor.matmul(
                            pacc[:, :512],
                            lhsT=wq_g[mc % 3][:, mc // 3, ks, :],
                            rhs=xlnT[th][:, ks, :],
                            start=(ks == 0), stop=(ks == CSUB - 1))
                    # bias+copy PSUM->SBUF (PSUM is DVE/ACT-only)
                    nc.vector.tensor_scalar_add(
                        qkvT[mc][:, th * 512:(th + 1) * 512], pacc[:, :512],
                        bqkv_sb[:, mc:mc + 1])

            def emit_vones(j):
                # vT row j -> v for heads 2j, 2j+1 (col 0 stays all-ones).
                # All 8 transposes land in one PSUM tile, drained by a single
                # Pool copy, so the PE rips through without ring round-trips.
                pt8 = ps_t.tile([P, NT, P], BF16, tag="pt8", name=f"vt{j}")
                for ti in range(NT):
                    nc.tensor.transpose(
                        pt8[:, ti, :],
                        qkvT[2 * (D6 // P) + j][:, ti * P:(ti + 1) * P],
                        ident[:])
                nc.vector.tensor_copy(
                    v_ones[:, :, 2 * j:2 * j + 2, 1:],
                    pt8[:].rearrange("p t (a b) -> p t a b", a=2))

            def emit_scores(h, pT):
                # scores transposed sT[k, q] so Exp lands pT in SBUF directly;
                # the causal mask of the diagonal block is added by the PE.
                qp0 = 64 * (h % 2)
                qrow = h // 2
                kp0 = (D6 + 64 * h) % P
                krow = (D6 + 64 * h) // P
                for kb in range(4):
                    q0 = kb * P
                    pscore = ps_sc.tile([P, T], F32, tag="sc",
                                        name=f"sc{h}{kb}")
                    # chunk on absolute 512 boundaries (PSUM bank alignment
                    # for the matmuls); Exp drains the whole row in one op
                    for (s0, e0) in ((q0, 512), (512, T)):
                        w = e0 - s0
                        nc.tensor.matmul(
                            pscore[:, s0:s0 + w],
                            lhsT=qkvT[krow][kp0:kp0 + 64, kb * P:(kb + 1) * P],
                            rhs=qkvT[qrow][qp0:qp0 + 64, s0:s0 + w],
                            start=True, stop=True)
                    nc.scalar.activation(
                        pT[:, kb, q0:], pscore[:, q0:], AF.Exp)
                    # causal mask of the diagonal block: 0/1 multiply on the
                    # near-idle Pool engine (SBUF-only op) instead of a PE
                    # matmul add — exp of the unmasked scores stays finite
                    # in bf16 (|s| < ~20)
                    nc.gpsimd.tensor_tensor(
                        pT[:, kb, q0:q0 + P], pT[:, kb, q0:q0 + P],
                        cm[:], op=ALU.mult)
                # kb 4..7 in pairs: both rows share one PSUM tile over the
                # union column range and drain with a single Exp — fewer ACT
                # ops and half the sc-ring slots. The younger row's leading
                # union columns are stale PSUM; their exp lands in pT cells
                # no AV ever reads (kb > qi there).
                for kp in range(2):
                    kb0 = 4 + 2 * kp
                    u0 = kb0 * P          # union start = older row's q0
                    uw = T - u0
                    pscore = ps_sc.tile([P, 2, 512], F32, tag="sc",
                                        name=f"sc{h}p{kp}")
                    for r in range(2):
                        kb = kb0 + r
                        q0 = kb * P
                        nc.tensor.matmul(
                            pscore[:, r, q0 - u0:uw],
                            lhsT=qkvT[krow][kp0:kp0 + 64, kb * P:(kb + 1) * P],
                            rhs=qkvT[qrow][qp0:qp0 + 64, q0:],
                            start=True, stop=True)
                    nc.scalar.activation(
                        pT[:, kb0:kb0 + 2, u0:], pscore[:, :, :uw], AF.Exp)
                    for r in range(2):
                        kb = kb0 + r
                        q0 = kb * P
                        nc.gpsimd.tensor_tensor(
                            pT[:, kb, q0:q0 + P], pT[:, kb, q0:q0 + P],
                            cm[:], op=ALU.mult)

            yT = [big.tile([P, D6 // P, T // 2], BF16, tag=f"yT{i}",
                           name=f"yT{i}") for i in range(2)]

            def emit_av(h, pT):
                # AV for head h, softmax denominator fused via v_ones col 0;
                # normalization batched per qi-pair on DVE
                for q2 in range(NT // 2):
                    py = ps_y.tile([P, 2, 65], F32, tag="py",
                                   name=f"py{h}{q2}")
                    for qq in range(2):
                        qi = 2 * q2 + qq
                        for kb in range(qi + 1):
                            nc.tensor.matmul(
                                py[:, qq, :],
                                lhsT=pT[:, kb, qi * P:(qi + 1) * P],
                                rhs=v_ones[:, kb, h, :],
                                start=(kb == 0), stop=(kb == qi))
                    rec = work.tile([P, 2], F32, tag="rec")
                    nc.vector.reciprocal(rec[:], py[:, :, 0:1])
                    nc.vector.tensor_tensor(
                        y_big[:, 2 * q2:2 * q2 + 2, h * 64:(h + 1) * 64],
                        py[:, :, 1:],
                        rec[:].to_broadcast([P, 2, 64]), op=ALU.mult)

            def emit_yt(g):
                # batched yT transpose for the 128-row block of heads 2g/2g+1
                pt8 = ps_t.tile([P, NT, P], BF16, tag="pt8", name=f"yt{g}")
                for qi in range(NT):
                    nc.tensor.transpose(
                        pt8[:, qi, :], y_big[:, qi, g * P:(g + 1) * P],
                        ident[:])
                for i in range(2):
                    nc.vector.tensor_copy(
                        yT[i][:, g, :], pt8[:, 4 * i:4 * i + 4, :]
                        .rearrange("p a b -> p (a b)"))

            # head-level software pipelining, depth 2: AV(h) trails
            # scores(h+2), so the exp stream on ACT (the middle-phase
            # critical path) never starves while the in-order PE waits on
            # exp-gated AV work
            pTs = []

            def emit_block(h):
                g = h // 2
                if h % 2 == 0:
                    emit_qkv(g, use_sc=(h == 0))
                    emit_qkv(3 + g, use_sc=(h == 0))
                else:
                    emit_qkv(6 + g)
                    emit_vones(g)
                pT = pTp.tile([P, NT, T], BF16, tag="pT", name=f"pT{h}")
                emit_scores(h, pT)
                pTs.append(pT)

            emit_block(0)
            emit_block(1)
            emit_block(2)
            for h in range(H6):
                if h + 3 < H6:
                    emit_block(h + 3)
                emit_av(h, pTs[h])
                if h % 2 == 1:
                    emit_yt(h // 2)

            # ---- partial projection: outT [C, T] = wpj.T @ y.T + bpj
            # pacc alternates between the two PSUM rings so chunks pipeline;
            # drains alternate DVE/ACT (exp stream is over; Identity shares
            # every act table so no reload); the last c-tile DMAs per half
            # so the final transfer is small
            for cc in range(CSUB):
                o_sb = work.tile([P, T], BF16, tag="osb")
                for th in range(T // 512):
                    pool = ps_mm if (2 * cc + th) % 2 == 0 else ps_sc
                    tag = "mm" if pool is ps_mm else "sc"
                    pacc = pool.tile([P, 512], F32, tag=tag,
                                     name=f"pj{cc}{th}")
                    for j in range(D6 // P):
                        nc.tensor.matmul(
                            pacc[:, :512],
                            lhsT=wpj_sb[:, j, cc * P:(cc + 1) * P],
                            rhs=yT[th][:, j, :],
                            start=(j == 0), stop=(j == D6 // P - 1))
                    if th == 0:
                        nc.vector.tensor_scalar_add(
                            o_sb[:, :512], pacc[:, :512],
                            bpj_sb[:, cc:cc + 1])
                    else:
                        nc.scalar.activation(
                            o_sb[:, 512:], pacc[:, :512], AF.Identity,
                            bias=bpj_sb[:, cc:cc + 1])
                    if cc == CSUB - 1:
                        nc.sync.dma_start(
                            out[cc * P:(cc + 1) * P,
                                th * 512:(th + 1) * 512],
                            o_sb[:, th * 512:(th + 1) * 512])
                if cc < CSUB - 1:
                    nc.sync.dma_start(out[cc * P:(cc + 1) * P, :], o_sb[:])

    nc.compile()
    return nc


# --------------------------------------------------------------------------
# Launch B: experts
# --------------------------------------------------------------------------

def build_expert(cap_k, grouped=True):
    """fp8(e4m3) expert MLP. Both matmuls run in DoubleRow perf mode (K=256
    per instruction, 0.5 cyc/row). Weights are quantized host-side with
    scales shared per 4-mf group; dequant rides the PSUM-drain op. hT stays
    fp8 in SBUF (mm2's rhs must be fp8). Token dim is processed in 256-col
    chunks, interleaving mm1/mm2 chunks on the (in-order) PE so mm2 work
    hides behind the gelu stream on ACT — which is the pacing engine.
    `grouped` (one ACT op per 4-mf [P,4,256] PSUM region) requires
    group-equal biases; fallback is one gelu per mf."""
    nc = bacc.Bacc("TRN2", target_bir_lowering=False, debug=False)

    G = 4                    # mf group per gelu op
    NQ = KSUB_F // G         # 6 groups
    xbT = nc.dram_tensor("xbT", [P, CSUB, cap_k], FP8, kind="ExternalInput")
    fcw = nc.dram_tensor("fcw", [P, KSUB_F * CSUB * P], FP8,
                         kind="ExternalInput")
    nsc = NQ if grouped else KSUB_F
    # packed [fcs | fcb | pjs | pjb] — one DMA
    scb = nc.dram_tensor("scb", [P, 2 * nsc + 2 * CSUB], F32,
                         kind="ExternalInput")
    pjw = nc.dram_tensor("pjw", [P, CSUB * KSUB_F * P], FP8,
                         kind="ExternalInput")
    out = nc.dram_tensor("outT", [C, cap_k], BF16, kind="ExternalOutput")

    SC = _chunks(cap_k, 256)          # compute chunks
    SD = _chunks(cap_k, 512)          # xbT DMA pieces (512B runs)
    DR = mybir.MatmulPerfMode.DoubleRow
    MFBLK = CSUB * P                  # 768 fp8 bytes per mf per partition

    NCH = len(SC)
    with tile.TileContext(nc) as tc:
        with (
            tc.tile_pool(name="const", bufs=1) as const,
            tc.tile_pool(name="osb", bufs=4) as osbp,
            tc.tile_pool(name="ps1", bufs=2, space="PSUM") as ps1,
            tc.tile_pool(name="ps2", bufs=3, space="PSUM") as ps2,
            tc.tile_pool(name="psw", bufs=1, space="PSUM") as psw,
        ):
            # PE warmup during the xbT/weight DMA lead-in
            wz = const.tile([P, 512], BF16, name="wz")
            nc.vector.memset(wz[:], 0.0)
            for wi in range(6):
                pw = psw.tile([P, 512], F32, tag="warm", name=f"warm{wi}")
                nc.tensor.matmul(pw[:], lhsT=wz[:, :P], rhs=wz[:],
                                 start=True, stop=True)

            # NOTE: tile-granular dependency tracking — every dma/compute
            # producer gets its own tile so consumers wait only on what they
            # actually read. One packed tile for the four tiny scale/bias
            # vectors (single DMA; HWDGE setup is ~0.6us per copy).
            sc_sb = const.tile([P, 2 * nsc + 2 * CSUB], F32)
            fcs_sb = sc_sb[:, :nsc]
            fcb_sb = sc_sb[:, nsc:2 * nsc]
            pjs_sb = sc_sb[:, 2 * nsc:2 * nsc + CSUB]
            pjb_sb = sc_sb[:, 2 * nsc + CSUB:]

            xb_t = [const.tile([P, CSUB, sw], FP8, name=f"xb{i}")
                    for i, (s0, sw) in enumerate(SD)]
            # quad 0 split in two 2-mf tiles: its DMA gates the very first
            # matmul, so halving the first transfer shaves the lead-in
            w1_q0 = [const.tile([P, 2, CSUB, P], FP8, name=f"w1q0{h}")
                     for h in range(2)]
            w1_t = [const.tile([P, G, CSUB, P], FP8, name=f"w1q{q}")
                    for q in range(1, NQ)]

            def w1_lhsT(q, g):
                if q == 0:
                    return w1_q0[g // 2][:, g % 2]
                return w1_t[q - 1][:, g]
            w2_t = [const.tile([P, KSUB_F, P], FP8, name=f"w2c{cc}")
                    for cc in range(CSUB)]
            hT_t = [const.tile([P, KSUB_F, sw], FP8, name=f"hT{i}")
                    for i, (s0, sw) in enumerate(SC)]

            def dma_xbT(i):
                s0, sw = SD[i]
                nc.sync.dma_start(xb_t[i][:], xbT[:, :, s0:s0 + sw])

            def dma_w1(q):
                if q == 0:
                    for h in range(2):
                        nc.sync.dma_start(
                            w1_q0[h][:].rearrange("p a b c -> p (a b c)"),
                            fcw[:, 2 * h * MFBLK:2 * (h + 1) * MFBLK])
                    return
                nc.sync.dma_start(
                    w1_t[q - 1][:].rearrange("p a b c -> p (a b c)"),
                    fcw[:, q * G * MFBLK:(q + 1) * G * MFBLK])

            def dma_w2(cc):
                blk = KSUB_F * P
                nc.sync.dma_start(
                    w2_t[cc][:].rearrange("p a b -> p (a b)"),
                    pjw[:, cc * blk:(cc + 1) * blk])

            # ordered by first consumer: xbT piece 0 and all w1 first (they
            # pace mm1(c0) and with it the whole gelu stream), then w2
            # staged around the late-needed xbT tail
            dma_xbT(0)
            dma_w1(0)
            nc.sync.dma_start(sc_sb[:], scb[:])
            for q in range(1, NQ):
                dma_w1(q)
            dma_w2(0)
            dma_w2(1)
            dma_w2(2)
            for i in range(1, len(SD)):
                dma_xbT(i)
            dma_w2(3)
            dma_w2(4)
            dma_w2(5)

            def piece_of(s0):
                for i, (p0, pw) in enumerate(SD):
                    if p0 <= s0 < p0 + pw:
                        return i, s0 - p0
                raise AssertionError(s0)

            def mm1(ci):
                s0, sw = SC[ci]
                di, d0 = piece_of(s0)
                for q in range(NQ):
                    if ci == 0:
                        # filler: the first chunk is paced by the w1 quad
                        # DMAs; keep the PE p-state warm across the ~0.4us
                        # per-quad wait
                        pw = psw.tile([P, 512], F32, tag="warm",
                                      name=f"fill{q}")
                        nc.tensor.matmul(pw[:], lhsT=wz[:, :P], rhs=wz[:],
                                         start=True, stop=True)
                    pacc = ps1.tile([P, G, 256], F32, tag="mm1")
                    for g in range(G):
                        for j in range(CSUB // 2):
                            nc.tensor.matmul(
                                pacc[:, g, :sw],
                                lhsT=w1_lhsT(q, g)[:, 2 * j:2 * j + 2, :],
                                rhs=xb_t[di][:, 2 * j:2 * j + 2, d0:d0 + sw],
                                start=(j == 0), stop=(j == CSUB // 2 - 1),
                                perf_mode=DR)
                    if grouped:
                        nc.scalar.activation(
                            hT_t[ci][:, G * q:G * q + G, :sw],
                            pacc[:, :, :sw], AF.Gelu,
                            bias=fcb_sb[:, q:q + 1],
                            scale=fcs_sb[:, q:q + 1])
                    else:
                        for g in range(G):
                            mf = G * q + g
                            nc.scalar.activation(
                                hT_t[ci][:, mf, :sw],
                                pacc[:, g, :sw], AF.Gelu,
                                bias=fcb_sb[:, mf:mf + 1],
                                scale=fcs_sb[:, mf:mf + 1])

            def mm2(ci, cc):
                # one (chunk, output-c-tile) piece: parks the in-order PE
                # only on its own w2 tile / hT chunk
                s0, sw = SC[ci]
                pacc = ps2.tile([P, 256], F32, tag="mm2")
                for j in range(KSUB_F // 2):
                    nc.tensor.matmul(
                        pacc[:, :sw],
                        lhsT=w2_t[cc][:, 2 * j:2 * j + 2, :],
                        rhs=hT_t[ci][:, 2 * j:2 * j + 2, :sw],
                        start=(j == 0), stop=(j == KSUB_F // 2 - 1),
                        perf_mode=DR)
                # dequant+bias on DVE; ACT is saturated by mm1's gelu
                o_sb = osbp.tile([P, 256], BF16, tag="osb")
                nc.vector.tensor_scalar(
                    o_sb[:, :sw], pacc[:, :sw],
                    pjs_sb[:, cc:cc + 1], pjb_sb[:, cc:cc + 1],
                    op0=ALU.mult, op1=ALU.add)
                nc.sync.dma_start(
                    out[cc * P:(cc + 1) * P, s0:s0 + sw], o_sb[:, :sw])

            # PE program order: the gelu stream on ACT (1.09us per quad-op)
            # paces the pipeline; mm1(c) quads are throttled to it by the
            # ps1 rotation, so weave the previous chunk's mm2 pieces into
            # the stream to keep the in-order PE from idling
            mm1(0)
            for i in range(1, NCH):
                mm1(i)
                for cc in range(CSUB):
                    mm2(i - 1, cc)
            for cc in range(CSUB):
                mm2(NCH - 1, cc)

    nc.compile()
    return nc


# --------------------------------------------------------------------------
# Host glue
# --------------------------------------------------------------------------

def _bf16(a):
    return np.asarray(a, np.float32).astype(ml_dtypes.bfloat16)


def _pcol(vec, nsub):
    """[nsub*P] -> [P, nsub] per-partition bias layout."""
    return np.ascontiguousarray(
        np.asarray(vec, np.float32).reshape(nsub, P).T)


def _kperm(w):
    """[K, N] -> [P, K//P, N] partition-major layout, contiguous."""
    k, n = w.shape
    return np.ascontiguousarray(w.reshape(k // P, P, n).transpose(1, 0, 2))


def _layer_norm(x, w, b):
    mu = x.mean(-1, keepdims=True)
    var = x.var(-1, keepdims=True)
    return (x - mu) / np.sqrt(var + LN_EPS) * w + b


def _exact_logits(need, x, ln1_w, ln1_b, ln2_w, ln2_b, qkv_w, qkv_b,
                  proj_w, proj_b, w_g):
    """fp32 gating logits for the given flat token indices (exact attention
    rows for just those tokens)."""
    out = np.empty((need.size, E), np.float32)
    bs, ps = need // T, need % T
    for b in np.unique(bs):
        m = bs == b
        pos = ps[m]                              # [M]
        xl = _layer_norm(x[b], ln1_w, ln1_b)     # [T, C]
        kv = xl @ qkv_w[:, C:] + qkv_b[C:]       # [T, 2C]
        k = kv[:, :C].reshape(T, NHEAD, HD)
        v = kv[:, C:].reshape(T, NHEAD, HD)
        q = (xl[pos] @ qkv_w[:, :C] + qkv_b[:C]).reshape(-1, NHEAD, HD)
        s = np.einsum("mhd,khd->mhk", q, k) / math.sqrt(HD)
        s = np.where(pos[:, None, None] >= np.arange(T)[None, None, :],
                     s, NEG_INF)
        s -= s.max(-1, keepdims=True)
        p = np.exp(s)
        p /= p.sum(-1, keepdims=True)
        y = np.einsum("mhk,khd->mhd", p, v).reshape(-1, C)
        att = y @ proj_w + proj_b
        x2 = x[b][pos] + att
        out[m] = _layer_norm(x2, ln2_w, ln2_b) @ w_g
    return out


def kernel(x, ln1_w, ln1_b, ln2_w, ln2_b, attn_qkv_w, attn_qkv_b,
           attn_proj_w, attn_proj_b, w_g, exp_fc_w, exp_fc_b,
           exp_proj_w, exp_proj_b):
    x = np.asarray(x, np.float32)
    ln1_w = np.asarray(ln1_w, np.float32)
    ln1_b = np.asarray(ln1_b, np.float32)
    attn_qkv_w = np.asarray(attn_qkv_w, np.float32)
    attn_qkv_b = np.asarray(attn_qkv_b, np.float32)
    attn_proj_w = np.asarray(attn_proj_w, np.float32)
    attn_proj_b = np.asarray(attn_proj_b, np.float32)

    if "attn" not in _CACHE:
        _CACHE["attn"] = build_attn()

    # ---------------- launch A ----------------
    # fold ln1 affine into qkv: qkv = xhat @ (diag(w1) W) + (b1 @ W + b)
    Wf = ln1_w[:, None] * attn_qkv_w          # [C, 3C]
    bf = ln1_b @ attn_qkv_w + attn_qkv_b      # [3C]
    Wq = Wf[:, :C] / math.sqrt(HD)
    bq = bf[:C] / math.sqrt(HD)
    Wk, bk = Wf[:, C:2 * C], bf[C:2 * C]
    Wv, bv = Wf[:, 2 * C:], bf[2 * C:]

    cmaskT_np = _bf16(np.triu(np.ones((P, P), np.float32)))

    in_maps_a = []
    for core in range(N_CORES):
        b = core // 2
        h0 = H6 * (core % 2)
        cols = slice(h0 * HD, (h0 + H6) * HD)
        wqkv_c = np.concatenate([Wq[:, cols], Wk[:, cols], Wv[:, cols]], 1)
        bqkv_c = np.concatenate([bq[cols], bk[cols], bv[cols]])
        bpj_c = attn_proj_b if core % 2 == 0 else np.zeros(C, np.float32)
        mu_b = x[b].mean(-1)
        rstd_b = 1.0 / np.sqrt(x[b].var(-1) + LN_EPS)
        xln_b = (x[b] - mu_b[:, None]) * rstd_b[:, None]    # [T, C]
        xlnT_h = _bf16(xln_b).T.reshape(CSUB, P, T).transpose(1, 0, 2)
        wqkv_m = _bf16(wqkv_c).reshape(CSUB, P, QKV9, P)
        wqkv_m = wqkv_m.transpose(1, 2, 0, 3)[:, [0, 3, 6, 1, 4, 7, 2, 5, 8]]
        meta = np.concatenate([
            _pcol(bqkv_c, QKV9), _pcol(bpj_c, CSUB)], axis=1)
        in_maps_a.append({
            "xlnT": np.ascontiguousarray(xlnT_h),
            "meta": np.ascontiguousarray(meta.astype(np.float32)),
            "wqkv": np.ascontiguousarray(wqkv_m.reshape(P, -1)),
            "wpj": _kperm(_bf16(attn_proj_w[h0 * HD:(h0 + H6) * HD, :])),
            "cmaskT": cmaskT_np,
        })

    res_a = _run_spmd(_CACHE["attn"], in_maps_a)

    attn = np.empty((B, T, C), np.float32)
    for b in range(B):
        attn[b] = (np.asarray(res_a.results[2 * b]["attn_pT"], np.float32)
                   + np.asarray(res_a.results[2 * b + 1]["attn_pT"],
                                np.float32)).T

    x2 = x + attn                       # [B, T, C]
    xf2 = x2.reshape(B * T, C)

    # ---------------- host routing (exact reference semantics) -------------
    N = B * T
    xln2 = _layer_norm(xf2, np.asarray(ln2_w, np.float32),
                       np.asarray(ln2_b, np.float32))
    logits = xln2 @ np.asarray(w_g, np.float32)        # [N, E]

    # The top-2 expert choice is discontinuous: tokens whose top2/top3 gating
    # logits are within the bf16 noise floor could route differently than the
    # fp32 reference would. Recompute those few tokens' logits exactly.
    srt = np.sort(logits, axis=1)
    need = np.nonzero(srt[:, -2] - srt[:, -3] < 0.02)[0]
    if need.size:
        logits[need] = _exact_logits(
            need, x, ln1_w, ln1_b, np.asarray(ln2_w, np.float32),
            np.asarray(ln2_b, np.float32), attn_qkv_w, attn_qkv_b,
            attn_proj_w, attn_proj_b, np.asarray(w_g, np.float32))

    order = np.argsort(-logits, axis=1, kind="stable")
    topk_idx = order[:, :TOPK]                          # [N, K]
    sel = np.zeros((N, E), bool)
    np.put_along_axis(sel, topk_idx, True, axis=1)
    masked = np.where(sel, logits, NEG_INF)
    m = masked.max(1, keepdims=True)
    ex = np.exp(masked - m)
    router_probs = ex / ex.sum(1, keepdims=True)        # [N, E]

    # capacity ranks in (k, n) order
    exp_mask = np.zeros((TOPK, N, E), np.int64)
    kk = np.arange(TOPK)[:, None]
    nn = np.arange(N)[None, :]
    exp_mask[kk, nn, topk_idx.T] = 1
    flat = exp_mask.reshape(TOPK * N, E)
    rank = np.cumsum(flat, axis=0) - 1                  # [K*N, E]
    keep = (flat == 1) & (rank < CAP)
    kpos, epos = np.nonzero(keep)
    token = kpos % N
    slot = rank[kpos, epos]
    wgt = router_probs[token, epos]

    # pack the expert batches to the observed max load; if only a few rows
    # push one expert past 1024 slots (= 2 full PSUM chunks), keep the device
    # batch at 1024 and run the leftover rows on the host in fp32.
    loads = np.bincount(epos, minlength=E)
    max_load = int(loads.max())
    cap_k64 = max(64, -(-max_load // 64) * 64)
    overflow = int(np.maximum(loads - 1024, 0).sum())
    cap_k = 1024 if (cap_k64 > 1024 and overflow <= 192) \
        else min(CAP, cap_k64)

    on_dev = slot < cap_k
    idx_e = np.zeros((E, cap_k), np.int64)
    w_e = np.zeros((E, cap_k), np.float32)
    idx_e[epos[on_dev], slot[on_dev]] = token[on_dev]
    w_e[epos[on_dev], slot[on_dev]] = wgt[on_dev]

    # ---------------- launch B ----------------
    # fp8(e4m3) quantization: activations cast directly (|xln2| ~ 4.7, well
    # inside e4m3 normal range); weights scaled to ~224 absmax (shared per
    # mf-pair so one gelu op can drain a 2-bank PSUM region), dequant folded
    # into the PSUM-drain ops on device.
    xln2_q8 = np.clip(xln2, -240, 240).astype(E4)
    exp_fc_w = np.asarray(exp_fc_w, np.float32)
    exp_fc_b = np.asarray(exp_fc_b, np.float32).reshape(E, F)
    exp_proj_w = np.asarray(exp_proj_w, np.float32)
    exp_proj_b = np.asarray(exp_proj_b, np.float32).reshape(E, C)

    G = 4
    fcb_r = exp_fc_b.reshape(E, KSUB_F // G, G, P)
    paired = bool((fcb_r == fcb_r[:, :, :1]).all())

    in_maps_b = []
    for e in range(E):
        xbT = _kperm(np.ascontiguousarray(xln2_q8[idx_e[e]].T))
        a1 = np.abs(exp_fc_w[e]).max(0).reshape(KSUB_F // G, G, P)
        if paired:
            gmax = a1.max(1)                                  # [6, p]
            s1g = 224.0 / np.maximum(gmax, 1e-30)
            s1 = np.repeat(s1g, G, axis=0).reshape(F)
            fcb_h = np.ascontiguousarray(fcb_r[e, :, 0].T)
        else:
            s1g = 224.0 / np.maximum(a1.reshape(KSUB_F, P), 1e-30)
            s1 = s1g.reshape(F)
            fcb_h = np.ascontiguousarray(fcb_r[e].reshape(KSUB_F, P).T)
        s2 = 224.0 / np.maximum(np.abs(exp_proj_w[e]).max(0), 1e-30)  # [C]
        fcw = np.clip(exp_fc_w[e] * s1, -240, 240).astype(E4)
        fcw = fcw.reshape(CSUB, P, KSUB_F, P).transpose(1, 2, 0, 3)
        pjw = np.clip(exp_proj_w[e] * s2, -240, 240).astype(E4)
        pjw = pjw.reshape(KSUB_F, P, CSUB, P).transpose(1, 2, 0, 3)
        scb = np.concatenate([
            (1.0 / s1g).T, fcb_h,
            _pcol(1.0 / s2, CSUB), _pcol(exp_proj_b[e], CSUB)], axis=1)
        in_maps_b.append({
            "xbT": xbT,
            "fcw": np.ascontiguousarray(fcw.reshape(P, -1)),
            "pjw": np.ascontiguousarray(pjw.reshape(P, -1)),
            "scb": np.ascontiguousarray(scb.astype(np.float32)),
        })

    if ("expert", cap_k, paired) not in _CACHE:
        _CACHE[("expert", cap_k, paired)] = build_expert(cap_k, paired)
    res_b = _run_spmd(_CACHE[("expert", cap_k, paired)], in_maps_b)

    y = xf2.copy()
    for e in range(E):
        valid = w_e[e] != 0
        outT = np.asarray(res_b.results[e]["outT"]).astype(np.float32)
        y[idx_e[e, valid]] += w_e[e, valid, None] * outT.T[valid]

    # host top-up for the few rows beyond cap_k (exact fp32)
    if not on_dev.all():
        try:
            from scipy.special import erf
        except ImportError:
            erf = np.vectorize(math.erf)
        off = ~on_dev
        for e in np.unique(epos[off]):
            m = off & (epos == e)
            tk = token[m]
            h = xln2[tk] @ exp_fc_w[e] + exp_fc_b[e]
            h = 0.5 * h * (1.0 + erf(h / math.sqrt(2.0)))
            o = h @ exp_proj_w[e] + exp_proj_b[e]
            y[tk] += wgt[m, None] * o
    return y.reshape(B, T, C).astype(np.float32)



# revision 65
# speedup vs baseline: 1.0242x; 1.0115x over previous
"""MoE transformer block on 8 Trainium2 cores.

Layer: x = x + attn(ln1(x)); x = x + moe(ln2(x)).
Shapes: B=4, T=1024, C=768, H=12 heads, E=8 experts, top-2, cap=1280, F=3072.

Distribution:
  Launch A (attention, bf16): core i -> batch i//2, heads 6*(i%2) .. +6.
    LN1 (affine folded into the QKV weights) is applied host-side and x-hat
    arrives pre-transposed [C, T]. The head loop is software-pipelined depth
    2 (AV of head h trails scores of head h+2) so the softmax-exp stream on
    the Activation engine — the middle-phase critical path — never starves.
    Each core emits a partial (6-head) projection, transposed [C, T] bf16;
    host sums the two half-head partials per batch and adds the residual.
  Host: ln2 + gating + exact top-2 capacity routing (numpy, matches the jax
    reference in ordering; near-tie tokens get exact fp32 logits), builds
    per-expert gather indices.
  Launch B (experts, fp8): core e -> expert e, slots packed to min(observed
    max load rounded to 64, 1024). Both matmuls run fp8(e4m3) DoubleRow
    (K=256/instr, 0.5 cyc/row); weights are quantized per 4-mf group
    host-side, activations are cast directly (|x|<5 fits e4m3), dequant
    rides the PSUM-drain ops. Token dim is chunked 4x256 with mm2 pieces
    woven into the mm1/gelu stream. outT [C, cap_k] bf16; host scatter-adds
    w * out into y and computes overflow slots (beyond cap_k) in fp32.
"""

import math

import numpy as np
import ml_dtypes

import concourse.bacc as bacc
import concourse.bass as bass
import concourse.mybir as mybir
import concourse.tile as tile
from concourse import bass_utils
from concourse.masks import make_identity

F32 = mybir.dt.float32
BF16 = mybir.dt.bfloat16
FP8 = mybir.dt.float8e4
E4 = ml_dtypes.float8_e4m3  # matches TRN float8e4 (max ±240)
AF = mybir.ActivationFunctionType
ALU = mybir.AluOpType
AX = mybir.AxisListType

B, T, C = 4, 1024, 768
NHEAD = 12
HD = C // NHEAD  # 64
E = 8
TOPK = 2
CAP = 1280
F = 4 * C  # 3072
LN_EPS = 1e-5
NEG_INF = -1e30
P = 128

N_CORES = 8
H6 = NHEAD // 2          # heads per core
D6 = H6 * HD             # 384
CSUB = C // P            # 6
KSUB_F = F // P          # 24
NT = T // P              # 8
QKV9 = 3 * D6 // P       # 9

_CACHE = {}


def _chunks(n, step=512):
    out = []
    s = 0
    while s < n:
        out.append((s, min(step, n - s)))
        s += step
    return out


def _run_spmd(nc, in_maps):
    """run_bass_kernel_spmd with one retry (transient NRT/axon failures)."""
    try:
        return bass_utils.run_bass_kernel_spmd(
            nc, in_maps, core_ids=list(range(N_CORES)))
    except Exception:
        import time as _time
        _time.sleep(2.0)
        return bass_utils.run_bass_kernel_spmd(
            nc, in_maps, core_ids=list(range(N_CORES)))


# --------------------------------------------------------------------------
# Launch A: attention
# --------------------------------------------------------------------------

def build_attn():
    nc = bacc.Bacc("TRN2", target_bir_lowering=False, debug=False)

    # LN1 is applied host-side (host already computes the stats); the kernel
    # receives x-hat transposed [C, T] so the qkv matmuls start immediately.
    xlnTd = nc.dram_tensor("xlnT", [P, CSUB, T], BF16, kind="ExternalInput")
    # qkv weight slice for this core's 6 heads, ln1-folded, q pre-scaled by
    # 1/sqrt(HD), grouped (0,3,6),(1,4,7),(2,5,8) — the order the head loop
    # consumes the 128-col groups. column order within n:
    # q h0..h5 | k h0..h5 | v h0..h5 (64 cols each head)
    wqkv = nc.dram_tensor("wqkv", [P, QKV9 * CSUB * P], BF16,
                          kind="ExternalInput")
    wpj = nc.dram_tensor("wpj", [P, D6 // P, C], BF16, kind="ExternalInput")
    # transposed causal 0/1 mask (bf16): cmaskT[k, q] = 1 if k <= q else 0
    cmaskT = nc.dram_tensor("cmaskT", [P, P], BF16, kind="ExternalInput")
    # packed [bqkv(9) | bpj(6)] — one DMA; HWDGE setup is ~0.6us per copy,
    # so copy count gates the lead-in
    meta = nc.dram_tensor("meta", [P, QKV9 + CSUB], F32, kind="ExternalInput")
    out = nc.dram_tensor("attn_pT", [C, T], BF16, kind="ExternalOutput")

    with tile.TileContext(nc) as tc:
        with (
            tc.tile_pool(name="const", bufs=1) as const,
            tc.tile_pool(name="big", bufs=1) as big,
            tc.tile_pool(name="pTp", bufs=5) as pTp,
            tc.tile_pool(name="work", bufs=4) as work,
            tc.tile_pool(name="ps_sc", bufs=2, space="PSUM") as ps_sc,
            tc.tile_pool(name="ps_mm", bufs=1, space="PSUM") as ps_mm,
            tc.tile_pool(name="ps_t", bufs=1, space="PSUM") as ps_t,
            tc.tile_pool(name="ps_y", bufs=2, space="PSUM") as ps_y,
        ):
            # PE warmup during the DMA lead-in (p-state ramp)
            wz = const.tile([P, 512], BF16, name="wz")
            nc.vector.memset(wz[:], 0.0)
            for wi in range(12):
                pw = ps_mm.tile([P, 512], F32, tag="mm", name=f"warm{wi}")
                nc.tensor.matmul(pw[:], lhsT=wz[:, :P], rhs=wz[:],
                                 start=True, stop=True)

            meta_sb = const.tile([P, QKV9 + CSUB], F32)
            bqkv_sb = meta_sb[:, 0:QKV9]
            bpj_sb = meta_sb[:, QKV9:]
            cm = const.tile([P, P], BF16)
            xlnT = [big.tile([P, CSUB, T // 2], BF16, name=f"xlnT{i}")
                    for i in range(2)]
            wq_g = [const.tile([P, 3, CSUB, P], BF16, name=f"wqg{gi}")
                    for gi in range(3)]

            def dma_wq(gi):
                blk = 3 * CSUB * P
                nc.sync.dma_start(
                    wq_g[gi][:].rearrange("p a b c -> p (a b c)"),
                    wqkv[:, gi * blk:(gi + 1) * blk])

            # xlnT half 0 and the first qkv weight group gate the first
            # matmul — everything else (meta: first drain ~9us, cm: first
            # mask ~17us) queues behind them
            nc.sync.dma_start(xlnT[0][:], xlnTd[:, :, 0:T // 2])
            dma_wq(0)
            nc.sync.dma_start(meta_sb[:], meta[:])
            nc.sync.dma_start(xlnT[1][:], xlnTd[:, :, T // 2:T])
            dma_wq(1)
            nc.sync.dma_start(cm[:], cmaskT[:])
            dma_wq(2)
            ident = const.tile([P, P], BF16)
            make_identity(nc, ident[:])
            wpj_sb = const.tile([P, D6 // P, C], BF16)
            nc.sync.dma_start(wpj_sb[:], wpj[:])

            # ---- qkvT [3*D6, T] = wqkv.T @ xln.T, + bias
            # one SBUF tile per 128-row group so consumers wait only on the
            # rows they read
            qkvT = [big.tile([P, T], BF16, tag=f"qkvT{mc}", name=f"qkvT{mc}")
                    for mc in range(QKV9)]
            v_ones = big.tile([P, NT, H6, 1 + 64], BF16)
            nc.vector.memset(v_ones[:, :, :, 0:1], 1.0)
            y_big = big.tile([P, NT, D6], BF16)

            def emit_qkv(mc, use_sc=False):
                for th in range(T // 512):
                    # before any scores exist the sc ring is idle — borrow
                    # it for every other early chunk so the mm ring's
                    # drain-read latency never bubbles the PE
                    if use_sc and th % 2 == 1:
                        pacc = ps_sc.tile([P, 512], F32, tag="sc",
                                          name=f"qk{mc}{th}")
                    else:
                        pacc = ps_mm.tile([P, 512], F32, tag="mm",
                                          name=f"qk{mc}{th}")
                    for ks in range(CSUB):
                        nc.tensor.matmul(
                            pacc[:, :512],
                            lhsT=wq_g[mc % 3][:, mc // 3, ks, :],
                            rhs=xlnT[th][:, ks, :],
                            start=(ks == 0), stop=(ks == CSUB - 1))
                    # bias+copy PSUM->SBUF (PSUM is DVE/ACT-only)
                    nc.vector.tensor_scalar_add(
                        qkvT[mc][:, th * 512:(th + 1) * 512], pacc[:, :512],
                        bqkv_sb[:, mc:mc + 1])

            def emit_vones(j):
                # vT row j -> v for heads 2j, 2j+1 (col 0 stays all-ones).
                # All 8 transposes land in one PSUM tile, drained by a single
                # Pool copy, so the PE rips through without ring round-trips.
                pt8 = ps_t.tile([P, NT, P], BF16, tag="pt8", name=f"vt{j}")
                for ti in range(NT):
                    nc.tensor.transpose(
                        pt8[:, ti, :],
                        qkvT[2 * (D6 // P) + j][:, ti * P:(ti + 1) * P],
                        ident[:])
                nc.vector.tensor_copy(
                    v_ones[:, :, 2 * j:2 * j + 2, 1:],
                    pt8[:].rearrange("p t (a b) -> p t a b", a=2))

            def emit_scores(h, pT):
                # scores transposed sT[k, q] so Exp lands pT in SBUF directly;
                # the causal mask of the diagonal block is added by the PE.
                qp0 = 64 * (h % 2)
                qrow = h // 2
                kp0 = (D6 + 64 * h) % P
                krow = (D6 + 64 * h) // P
                for kb in range(4):
                    q0 = kb * P
                    pscore = ps_sc.tile([P, T], F32, tag="sc",
                                        name=f"sc{h}{kb}")
                    # chunk on absolute 512 boundaries (PSUM bank alignment
                    # for the matmuls); Exp drains the whole row in one op
                    for (s0, e0) in ((q0, 512), (512, T)):
                        w = e0 - s0
                        nc.tensor.matmul(
                            pscore[:, s0:s0 + w],
                            lhsT=qkvT[krow][kp0:kp0 + 64, kb * P:(kb + 1) * P],
                            rhs=qkvT[qrow][qp0:qp0 + 64, s0:s0 + w],
                            start=True, stop=True)
                    nc.scalar.activation(
                        pT[:, kb, q0:], pscore[:, q0:], AF.Exp)
                    # causal mask of the diagonal block: 0/1 multiply on the
                    # near-idle Pool engine (SBUF-only op) instead of a PE
                    # matmul add — exp of the unmasked scores stays finite
                    # in bf16 (|s| < ~20)
                    nc.gpsimd.tensor_tensor(
                        pT[:, kb, q0:q0 + P], pT[:, kb, q0:q0 + P],
                        cm[:], op=ALU.mult)
                # kb 4..7 in pairs: both rows share one PSUM tile over the
                # union column range and drain with a single Exp — fewer ACT
                # ops and half the sc-ring slots. The younger row's leading
                # union columns are stale PSUM; their exp lands in pT cells
                # no AV ever reads (kb > qi there).
                for kp in range(2):
                    kb0 = 4 + 2 * kp
                    u0 = kb0 * P          # union start = older row's q0
                    uw = T - u0
                    pscore = ps_sc.tile([P, 2, 512], F32, tag="sc",
                                        name=f"sc{h}p{kp}")
                    for r in range(2):
                        kb = kb0 + r
                        q0 = kb * P
                        nc.tensor.matmul(
                            pscore[:, r, q0 - u0:uw],
                            lhsT=qkvT[krow][kp0:kp0 + 64, kb * P:(kb + 1) * P],
                            rhs=qkvT[qrow][qp0:qp0 + 64, q0:],
                            start=True, stop=True)
                    nc.scalar.activation(
                        pT[:, kb0:kb0 + 2, u0:], pscore[:, :, :uw], AF.Exp)
                    for r in range(2):
                        kb = kb0 + r
                        q0 = kb * P
                        nc.gpsimd.tensor_tensor(
                            pT[:, kb, q0:q0 + P], pT[:, kb, q0:q0 + P],
                            cm[:], op=ALU.mult)

            yT = [big.tile([P, D6 // P, T // 2], BF16, tag=f"yT{i}",
                           name=f"yT{i}") for i in range(2)]

            def emit_av(h, pT):
                # AV for head h, softmax denominator fused via v_ones col 0;
                # normalization batched per qi-pair on DVE
                for q2 in range(NT // 2):
                    py = ps_y.tile([P, 2, 65], F32, tag="py",
                                   name=f"py{h}{q2}")
                    for qq in range(2):
                        qi = 2 * q2 + qq
                        for kb in range(qi + 1):
                            nc.tensor.matmul(
                                py[:, qq, :],
                                lhsT=pT[:, kb, qi * P:(qi + 1) * P],
                                rhs=v_ones[:, kb, h, :],
                                start=(kb == 0), stop=(kb == qi))
                    rec = work.tile([P, 2], F32, tag="rec")
                    nc.vector.reciprocal(rec[:], py[:, :, 0:1])
                    nc.vector.tensor_tensor(
                        y_big[:, 2 * q2:2 * q2 + 2, h * 64:(h + 1) * 64],
                        py[:, :, 1:],
                        rec[:].to_broadcast([P, 2, 64]), op=ALU.mult)

            def emit_yt(g):
                # batched yT transpose for the 128-row block of heads 2g/2g+1
                pt8 = ps_t.tile([P, NT, P], BF16, tag="pt8", name=f"yt{g}")
                for qi in range(NT):
                    nc.tensor.transpose(
                        pt8[:, qi, :], y_big[:, qi, g * P:(g + 1) * P],
                        ident[:])
                for i in range(2):
                    nc.vector.tensor_copy(
                        yT[i][:, g, :], pt8[:, 4 * i:4 * i + 4, :]
                        .rearrange("p a b -> p (a b)"))

            # head-level software pipelining, depth 2: AV(h) trails
            # scores(h+2), so the exp stream on ACT (the middle-phase
            # critical path) never starves while the in-order PE waits on
            # exp-gated AV work
            pTs = []

            def emit_block(h):
                g = h // 2
                if h % 2 == 0:
                    emit_qkv(g, use_sc=(h == 0))
                    emit_qkv(3 + g, use_sc=(h == 0))
                else:
                    emit_qkv(6 + g)
                    emit_vones(g)
                pT = pTp.tile([P, NT, T], BF16, tag="pT", name=f"pT{h}")
                emit_scores(h, pT)
                pTs.append(pT)

            emit_block(0)
            emit_block(1)
            emit_block(2)
            for h in range(H6):
                if h + 3 < H6:
                    emit_block(h + 3)
                emit_av(h, pTs[h])
                if h % 2 == 1:
                    emit_yt(h // 2)

            # ---- partial projection: outT [C, T] = wpj.T @ y.T + bpj
            # pacc alternates between the two PSUM rings so chunks pipeline;
            # drains alternate DVE/ACT (exp stream is over; Identity shares
            # every act table so no reload); the last c-tile DMAs per half
            # so the final transfer is small
            for cc in range(CSUB):
                o_sb = work.tile([P, T], BF16, tag="osb")
                for th in range(T // 512):
                    pool = ps_mm if (2 * cc + th) % 2 == 0 else ps_sc
                    tag = "mm" if pool is ps_mm else "sc"
                    pacc = pool.tile([P, 512], F32, tag=tag,
                                     name=f"pj{cc}{th}")
                    for j in range(D6 // P):
                        nc.tensor.matmul(
                            pacc[:, :512],
                            lhsT=wpj_sb[:, j, cc * P:(cc + 1) * P],
                            rhs=yT[th][:, j, :],
                            start=(j == 0), stop=(j == D6 // P - 1))
                    if th == 0:
                        nc.vector.tensor_scalar_add(
                            o_sb[:, :512], pacc[:, :512],
                            bpj_sb[:, cc:cc + 1])
                    else:
                        nc.scalar.activation(
                            o_sb[:, 512:], pacc[:, :512], AF.Identity,
                            bias=bpj_sb[:, cc:cc + 1])
                    if cc == CSUB - 1:
                        nc.sync.dma_start(
                            out[cc * P:(cc + 1) * P,
                                th * 512:(th + 1) * 512],
                            o_sb[:, th * 512:(th + 1) * 512])
                if cc < CSUB - 1:
                    nc.sync.dma_start(out[cc * P:(cc + 1) * P, :], o_sb[:])

    nc.compile()
    return nc


# --------------------------------------------------------------------------
# Launch B: experts
# --------------------------------------------------------------------------

def build_expert(cap_k, grouped=True):
    """fp8(e4m3) expert MLP. Both matmuls run in DoubleRow perf mode (K=256
    per instruction, 0.5 cyc/row). Weights are quantized host-side with
    scales shared per 4-mf group; dequant rides the PSUM-drain op. hT stays
    fp8 in SBUF (mm2's rhs must be fp8). Token dim is processed in 256-col
    chunks, interleaving mm1/mm2 chunks on the (in-order) PE so mm2 work
    hides behind the gelu stream on ACT — which is the pacing engine.
    `grouped` (one ACT op per 4-mf [P,4,256] PSUM region) requires
    group-equal biases; fallback is one gelu per mf."""
    nc = bacc.Bacc("TRN2", target_bir_lowering=False, debug=False)

    G = 4                    # mf group per gelu op
    NQ = KSUB_F // G         # 6 groups
    xbT = nc.dram_tensor("xbT", [P, CSUB, cap_k], FP8, kind="ExternalInput")
    fcw = nc.dram_tensor("fcw", [P, KSUB_F * CSUB * P], FP8,
                         kind="ExternalInput")
    nsc = NQ if grouped else KSUB_F
    # packed [fcs | fcb | pjs | pjb] — one DMA
    scb = nc.dram_tensor("scb", [P, 2 * nsc + 2 * CSUB], F32,
                         kind="ExternalInput")
    pjw = nc.dram_tensor("pjw", [P, CSUB * KSUB_F * P], FP8,
                         kind="ExternalInput")
    out = nc.dram_tensor("outT", [C, cap_k], BF16, kind="ExternalOutput")

    SC = _chunks(cap_k, 256)          # compute chunks
    SD = _chunks(cap_k, 512)          # xbT DMA pieces (512B runs)
    DR = mybir.MatmulPerfMode.DoubleRow
    MFBLK = CSUB * P                  # 768 fp8 bytes per mf per partition

    NCH = len(SC)
    with tile.TileContext(nc) as tc:
        with (
            tc.tile_pool(name="const", bufs=1) as const,
            tc.tile_pool(name="osb", bufs=4) as osbp,
            tc.tile_pool(name="ps1", bufs=2, space="PSUM") as ps1,
            tc.tile_pool(name="ps2", bufs=3, space="PSUM") as ps2,
            tc.tile_pool(name="psw", bufs=1, space="PSUM") as psw,
        ):
            # PE warmup during the xbT/weight DMA lead-in
            wz = const.tile([P, 512], BF16, name="wz")
            nc.vector.memset(wz[:], 0.0)
            for wi in range(6):
                pw = psw.tile([P, 512], F32, tag="warm", name=f"warm{wi}")
                nc.tensor.matmul(pw[:], lhsT=wz[:, :P], rhs=wz[:],
                                 start=True, stop=True)

            # NOTE: tile-granular dependency tracking — every dma/compute
            # producer gets its own tile so consumers wait only on what they
            # actually read. One packed tile for the four tiny scale/bias
            # vectors (single DMA; HWDGE setup is ~0.6us per copy).
            sc_sb = const.tile([P, 2 * nsc + 2 * CSUB], F32)
            fcs_sb = sc_sb[:, :nsc]
            fcb_sb = sc_sb[:, nsc:2 * nsc]
            pjs_sb = sc_sb[:, 2 * nsc:2 * nsc + CSUB]
            pjb_sb = sc_sb[:, 2 * nsc + CSUB:]

            xb_t = [const.tile([P, CSUB, sw], FP8, name=f"xb{i}")
                    for i, (s0, sw) in enumerate(SD)]
            # quad 0 split in two 2-mf tiles: its DMA gates the very first
            # matmul, so halving the first transfer shaves the lead-in
            w1_q0 = [const.tile([P, 2, CSUB, P], FP8, name=f"w1q0{h}")
                     for h in range(2)]
            w1_t = [const.tile([P, G, CSUB, P], FP8, name=f"w1q{q}")
                    for q in range(1, NQ)]

            def w1_lhsT(q, g):
                if q == 0:
                    return w1_q0[g // 2][:, g % 2]
                return w1_t[q - 1][:, g]
            w2_t = [const.tile([P, KSUB_F, P], FP8, name=f"w2c{cc}")
                    for cc in range(CSUB)]
            hT_t = [const.tile([P, KSUB_F, sw], FP8, name=f"hT{i}")
                    for i, (s0, sw) in enumerate(SC)]

            def dma_xbT(i):
                s0, sw = SD[i]
                nc.sync.dma_start(xb_t[i][:], xbT[:, :, s0:s0 + sw])

            def dma_w1(q):
                if q == 0:
                    for h in range(2):
                        nc.sync.dma_start(
                            w1_q0[h][:].rearrange("p a b c -> p (a b c)"),
                            fcw[:, 2 * h * MFBLK:2 * (h + 1) * MFBLK])
                    return
                nc.sync.dma_start(
                    w1_t[q - 1][:].rearrange("p a b c -> p (a b c)"),
                    fcw[:, q * G * MFBLK:(q + 1) * G * MFBLK])

            def dma_w2(cc):
                blk = KSUB_F * P
                nc.sync.dma_start(
                    w2_t[cc][:].rearrange("p a b -> p (a b)"),
                    pjw[:, cc * blk:(cc + 1) * blk])

            # ordered by first consumer: xbT piece 0 and all w1 first (they
            # pace mm1(c0) and with it the whole gelu stream), then w2
            # staged around the late-needed xbT tail
            dma_xbT(0)
            dma_w1(0)
            nc.sync.dma_start(sc_sb[:], scb[:])
            for q in range(1, NQ):
                dma_w1(q)
            dma_w2(0)
            dma_w2(1)
            dma_w2(2)
            for i in range(1, len(SD)):
                dma_xbT(i)
            dma_w2(3)
            dma_w2(4)
            dma_w2(5)

            def piece_of(s0):
                for i, (p0, pw) in enumerate(SD):
                    if p0 <= s0 < p0 + pw:
                        return i, s0 - p0
                raise AssertionError(s0)

            def mm1(ci):
                s0, sw = SC[ci]
                di, d0 = piece_of(s0)
                for q in range(NQ):
                    if ci == 0:
                        # filler: the first chunk is paced by the w1 quad
                        # DMAs; keep the PE p-state warm across the ~0.4us
                        # per-quad wait
                        pw = psw.tile([P, 512], F32, tag="warm",
                                      name=f"fill{q}")
                        nc.tensor.matmul(pw[:], lhsT=wz[:, :P], rhs=wz[:],
                                         start=True, stop=True)
                    pacc = ps1.tile([P, G, 256], F32, tag="mm1")
                    for g in range(G):
                        for j in range(CSUB // 2):
                            nc.tensor.matmul(
                                pacc[:, g, :sw],
                                lhsT=w1_lhsT(q, g)[:, 2 * j:2 * j + 2, :],
                                rhs=xb_t[di][:, 2 * j:2 * j + 2, d0:d0 + sw],
                                start=(j == 0), stop=(j == CSUB // 2 - 1),
                                perf_mode=DR)
                    if grouped:
                        nc.scalar.activation(
                            hT_t[ci][:, G * q:G * q + G, :sw],
                            pacc[:, :, :sw], AF.Gelu,
                            bias=fcb_sb[:, q:q + 1],
                            scale=fcs_sb[:, q:q + 1])
                    else:
                        for g in range(G):
                            mf = G * q + g
                            nc.scalar.activation(
                                hT_t[ci][:, mf, :sw],
                                pacc[:, g, :sw], AF.Gelu,
                                bias=fcb_sb[:, mf:mf + 1],
                                scale=fcs_sb[:, mf:mf + 1])

            def mm2(ci, cc):
                # one (chunk, output-c-tile) piece: parks the in-order PE
                # only on its own w2 tile / hT chunk
                s0, sw = SC[ci]
                pacc = ps2.tile([P, 256], F32, tag="mm2")
                for j in range(KSUB_F // 2):
                    nc.tensor.matmul(
                        pacc[:, :sw],
                        lhsT=w2_t[cc][:, 2 * j:2 * j + 2, :],
                        rhs=hT_t[ci][:, 2 * j:2 * j + 2, :sw],
                        start=(j == 0), stop=(j == KSUB_F // 2 - 1),
                        perf_mode=DR)
                # dequant+bias on DVE; ACT is saturated by mm1's gelu
                o_sb = osbp.tile([P, 256], BF16, tag="osb")
                nc.vector.tensor_scalar(
                    o_sb[:, :sw], pacc[:, :sw],
                    pjs_sb[:, cc:cc + 1], pjb_sb[:, cc:cc + 1],
                    op0=ALU.mult, op1=ALU.add)
                nc.sync.dma_start(
                    out[cc * P:(cc + 1) * P, s0:s0 + sw], o_sb[:, :sw])

            # PE program order: the gelu stream on ACT (1.09us per quad-op)
            # paces the pipeline; mm1(c) quads are throttled to it by the
            # ps1 rotation, so weave the previous chunk's mm2 pieces into
            # the stream to keep the in-order PE from idling
            mm1(0)
            for i in range(1, NCH):
                mm1(i)
                for cc in range(CSUB):
                    mm2(i - 1, cc)
            for cc in range(CSUB):
                mm2(NCH - 1, cc)

    nc.compile()
    return nc


# --------------------------------------------------------------------------
# Host glue
# --------------------------------------------------------------------------

def _bf16(a):
    return np.asarray(a, np.float32).astype(ml_dtypes.bfloat16)


def _pcol(vec, nsub):
    """[nsub*P] -> [P, nsub] per-partition bias layout."""
    return np.ascontiguousarray(
        np.asarray(vec, np.float32).reshape(nsub, P).T)


def _kperm(w):
    """[K, N] -> [P, K//P, N] partition-major layout, contiguous."""
    k, n = w.shape
    return np.ascontiguousarray(w.reshape(k // P, P, n).transpose(1, 0, 2))


def _layer_norm(x, w, b):
    mu = x.mean(-1, keepdims=True)
    var = x.var(-1, keepdims=True)
    return (x - mu) / np.sqrt(var + LN_EPS) * w + b


def _exact_logits(need, x, ln1_w, ln1_b, ln2_w, ln2_b, qkv_w, qkv_b,
                  proj_w, proj_b, w_g):
    """fp32 gating logits for the given flat token indices (exact attention
    rows for just those tokens)."""
    out = np.empty((need.size, E), np.float32)
    bs, ps = need // T, need % T
    for b in np.unique(bs):
        m = bs == b
        pos = ps[m]                              # [M]
        xl = _layer_norm(x[b], ln1_w, ln1_b)     # [T, C]
        kv = xl @ qkv_w[:, C:] + qkv_b[C:]       # [T, 2C]
        k = kv[:, :C].reshape(T, NHEAD, HD)
        v = kv[:, C:].reshape(T, NHEAD, HD)
        q = (xl[pos] @ qkv_w[:, :C] + qkv_b[:C]).reshape(-1, NHEAD, HD)
        s = np.einsum("mhd,khd->mhk", q, k) / math.sqrt(HD)
        s = np.where(pos[:, None, None] >= np.arange(T)[None, None, :],
                     s, NEG_INF)
        s -= s.max(-1, keepdims=True)
        p = np.exp(s)
        p /= p.sum(-1, keepdims=True)
        y = np.einsum("mhk,khd->mhd", p, v).reshape(-1, C)
        att = y @ proj_w + proj_b
        x2 = x[b][pos] + att
        out[m] = _layer_norm(x2, ln2_w, ln2_b) @ w_g
    return out


def kernel(x, ln1_w, ln1_b, ln2_w, ln2_b, attn_qkv_w, attn_qkv_b,
           attn_proj_w, attn_proj_b, w_g, exp_fc_w, exp_fc_b,
           exp_proj_w, exp_proj_b):
    x = np.asarray(x, np.float32)
    ln1_w = np.asarray(ln1_w, np.float32)
    ln1_b = np.asarray(ln1_b, np.float32)
    attn_qkv_w = np.asarray(attn_qkv_w, np.float32)
    attn_qkv_b = np.asarray(attn_qkv_b, np.float32)
    attn_proj_w = np.asarray(attn_proj_w, np.float32)
    attn_proj_b = np.asarray(attn_proj_b, np.float32)

    if "attn" not in _CACHE:
        _CACHE["attn"] = build_attn()

    # ---------------- launch A ----------------
    # fold ln1 affine into qkv: qkv = xhat @ (diag(w1) W) + (b1 @ W + b)
    Wf = ln1_w[:, None] * attn_qkv_w          # [C, 3C]
    bf = ln1_b @ attn_qkv_w + attn_qkv_b      # [3C]
    Wq = Wf[:, :C] / math.sqrt(HD)
    bq = bf[:C] / math.sqrt(HD)
    Wk, bk = Wf[:, C:2 * C], bf[C:2 * C]
    Wv, bv = Wf[:, 2 * C:], bf[2 * C:]

    cmaskT_np = _bf16(np.triu(np.ones((P, P), np.float32)))

    in_maps_a = []
    for core in range(N_CORES):
        b = core // 2
        h0 = H6 * (core % 2)
        cols = slice(h0 * HD, (h0 + H6) * HD)
        wqkv_c = np.concatenate([Wq[:, cols], Wk[:, cols], Wv[:, cols]], 1)
        bqkv_c = np.concatenate([bq[cols], bk[cols], bv[cols]])
        bpj_c = attn_proj_b if core % 2 == 0 else np.zeros(C, np.float32)
        mu_b = x[b].mean(-1)
        rstd_b = 1.0 / np.sqrt(x[b].var(-1) + LN_EPS)
        xln_b = (x[b] - mu_b[:, None]) * rstd_b[:, None]    # [T, C]
        xlnT_h = _bf16(xln_b).T.reshape(CSUB, P, T).transpose(1, 0, 2)
        wqkv_m = _bf16(wqkv_c).reshape(CSUB, P, QKV9, P)
        wqkv_m = wqkv_m.transpose(1, 2, 0, 3)[:, [0, 3, 6, 1, 4, 7, 2, 5, 8]]
        meta = np.concatenate([
            _pcol(bqkv_c, QKV9), _pcol(bpj_c, CSUB)], axis=1)
        in_maps_a.append({
            "xlnT": np.ascontiguousarray(xlnT_h),
            "meta": np.ascontiguousarray(meta.astype(np.float32)),
            "wqkv": np.ascontiguousarray(wqkv_m.reshape(P, -1)),
            "wpj": _kperm(_bf16(attn_proj_w[h0 * HD:(h0 + H6) * HD, :])),
            "cmaskT": cmaskT_np,
        })

    res_a = _run_spmd(_CACHE["attn"], in_maps_a)

    attn = np.empty((B, T, C), np.float32)
    for b in range(B):
        attn[b] = (np.asarray(res_a.results[2 * b]["attn_pT"], np.float32)
                   + np.asarray(res_a.results[2 * b + 1]["attn_pT"],
                                np.float32)).T

    x2 = x + attn                       # [B, T, C]
    xf2 = x2.reshape(B * T, C)

    # ---------------- host routing (exact reference semantics) -------------
    N = B * T
    xln2 = _layer_norm(xf2, np.asarray(ln2_w, np.float32),
                       np.asarray(ln2_b, np.float32))
    logits = xln2 @ np.asarray(w_g, np.float32)        # [N, E]

    # The top-2 expert choice is discontinuous: tokens whose top2/top3 gating
    # logits are within the bf16 noise floor could route differently than the
    # fp32 reference would. Recompute those few tokens' logits exactly.
    srt = np.sort(logits, axis=1)
    need = np.nonzero(srt[:, -2] - srt[:, -3] < 0.02)[0]
    if need.size:
        logits[need] = _exact_logits(
            need, x, ln1_w, ln1_b, np.asarray(ln2_w, np.float32),
            np.asarray(ln2_b, np.float32), attn_qkv_w, attn_qkv_b,
            attn_proj_w, attn_proj_b, np.asarray(w_g, np.float32))

    order = np.argsort(-logits, axis=1, kind="stable")
    topk_idx = order[:, :TOPK]                          # [N, K]
    sel = np.zeros((N, E), bool)
    np.put_along_axis(sel, topk_idx, True, axis=1)
    masked = np.where(sel, logits, NEG_INF)
    m = masked.max(1, keepdims=True)
    ex = np.exp(masked - m)
    router_probs = ex / ex.sum(1, keepdims=True)        # [N, E]

    # capacity ranks in (k, n) order
    exp_mask = np.zeros((TOPK, N, E), np.int64)
    kk = np.arange(TOPK)[:, None]
    nn = np.arange(N)[None, :]
    exp_mask[kk, nn, topk_idx.T] = 1
    flat = exp_mask.reshape(TOPK * N, E)
    rank = np.cumsum(flat, axis=0) - 1                  # [K*N, E]
    keep = (flat == 1) & (rank < CAP)
    kpos, epos = np.nonzero(keep)
    token = kpos % N
    slot = rank[kpos, epos]
    wgt = router_probs[token, epos]

    # pack the expert batches to the observed max load; if only a few rows
    # push one expert past 1024 slots (= 2 full PSUM chunks), keep the device
    # batch at 1024 and run the leftover rows on the host in fp32.
    loads = np.bincount(epos, minlength=E)
    max_load = int(loads.max())
    cap_k64 = max(64, -(-max_load // 64) * 64)
    overflow = int(np.maximum(loads - 1024, 0).sum())
    cap_k = 1024 if (cap_k64 > 1024 and overflow <= 192) \
        else min(CAP, cap_k64)

    on_dev = slot < cap_k
    idx_e = np.zeros((E, cap_k), np.int64)
    w_e = np.zeros((E, cap_k), np.float32)
    idx_e[epos[on_dev], slot[on_dev]] = token[on_dev]
    w_e[epos[on_dev], slot[on_dev]] = wgt[on_dev]

    # ---------------- launch B ----------------
    # fp8(e4m3) quantization: activations cast directly (|xln2| ~ 4.7, well
    # inside e4m3 normal range); weights scaled to ~224 absmax (shared per
    # mf-pair so one gelu op can drain a 2-bank PSUM region), dequant folded
    # into the PSUM-drain ops on device.
    xln2_q8 = np.clip(xln2, -240, 240).astype(E4)
    exp_fc_w = np.asarray(exp_fc_w, np.float32)
    exp_fc_b = np.asarray(exp_fc_b, np.float32).reshape(E, F)
    exp_proj_w = np.asarray(exp_proj_w, np.float32)
    exp_proj_b = np.asarray(exp_proj_b, np.float32).reshape(E, C)

    G = 4
    fcb_r = exp_fc_b.reshape(E, KSUB_F // G, G, P)
    paired = bool((fcb_r == fcb_r[:, :, :1]).all())

    in_maps_b = []
    for e in range(E):
        xbT = _kperm(np.ascontiguousarray(xln2_q8[idx_e[e]].T))
        a1 = np.abs(exp_fc_w[e]).max(0).reshape(KSUB_F // G, G, P)
        if paired:
            gmax = a1.max(1)                                  # [6, p]
            s1g = 224.0 / np.maximum(gmax, 1e-30)
            s1 = np.repeat(s1g, G, axis=0).reshape(F)
            fcb_h = np.ascontiguousarray(fcb_r[e, :, 0].T)
        else:
            s1g = 224.0 / np.maximum(a1.reshape(KSUB_F, P), 1e-30)
            s1 = s1g.reshape(F)
            fcb_h = np.ascontiguousarray(fcb_r[e].reshape(KSUB_F, P).T)
        s2 = 224.0 / np.maximum(np.abs(exp_proj_w[e]).max(0), 1e-30)  # [C]
        fcw = np.clip(exp_fc_w[e] * s1, -240, 240).astype(E4)
        fcw = fcw.reshape(CSUB, P, KSUB_F, P).transpose(1, 2, 0, 3)
        pjw = np.clip(exp_proj_w[e] * s2, -240, 240).astype(E4)
        pjw = pjw.reshape(KSUB_F, P, CSUB, P).transpose(1, 2, 0, 3)
        scb = np.concatenate([
            (1.0 / s1g).T, fcb_h,
            _pcol(1.0 / s2, CSUB), _pcol(exp_proj_b[e], CSUB)], axis=1)
        in_maps_b.append({
            "xbT": xbT,
            "fcw": np.ascontiguousarray(fcw.reshape(P, -1)),
            "pjw": np.ascontiguousarray(pjw.reshape(P, -1)),
            "scb": np.ascontiguousarray(scb.astype(np.float32)),
        })

    if ("expert", cap_k, paired) not in _CACHE:
        _CACHE[("expert", cap_k, paired)] = build_expert(cap_k, paired)
    res_b = _run_spmd(_CACHE[("expert", cap_k, paired)], in_maps_b)

    y = xf2.copy()
    for e in range(E):
        valid = w_e[e] != 0
        outT = np.asarray(res_b.results[e]["outT"]).astype(np.float32)
        y[idx_e[e, valid]] += w_e[e, valid, None] * outT.T[valid]

    # host top-up for the few rows beyond cap_k (exact fp32)
    if not on_dev.all():
        try:
            from scipy.special import erf
        except ImportError:
            erf = np.vectorize(math.erf)
        off = ~on_dev
        for e in np.unique(epos[off]):
            m = off & (epos == e)
            tk = token[m]
            h = xln2[tk] @ exp_fc_w[e] + exp_fc_b[e]
            h = 0.5 * h * (1.0 + erf(h / math.sqrt(2.0)))
            o = h @ exp_proj_w[e] + exp_proj_b[e]
            y[tk] += wgt[m, None] * o
    return y.reshape(B, T, C).astype(np.float32)

